# revision 1
# baseline (speedup 1.0000x reference)
"""Trainium2 Bass kernel for MultiHeadAttention with RoPE.

Problem: B=2, L=2048, d_model=1024, 16 heads, d_k=64, fp32 in/out.

Sharding (8 cores): tensor-parallel over heads — core c owns heads
{2c, 2c+1}, i.e. a 128-wide slice of the projection output dims.  Every
core reads the full q/k/v activations (transposed + bf16 on host), its
own 128-row slice of Wq/Wk/Wv (pre-transposed) and the matching 128
columns of Wo.  Each core computes its heads' attention output and the
partial d_model-sized output projection; the host sums the 8 partials
and adds bo.

Per-core pipeline (all matmuls bf16, fp32 PSUM accumulation):
  1. QKV projections:  qh.T = WqT.T @ q.T  laid out [128 head-dims, 4096 tok]
  2. RoPE on q,k via partition-shifted DMA copy + 3 DVE ops; the 1/sqrt(dk)
     scale and the rotate-half sign are folded into host-built cos/sin tables
  3. scores.T tiles [kt 128, qt 512] = kh'' (stationary, K=64) @ qh''
  4. exp on ScalarE (no max-subtract: scores ~ N(0,1), fp32 exp is safe),
     output bf16 -> SBUF
  5. ctx accumulation [65, qt]: stationary vh_aug [kt, 64 dims + ones col]
     -> row 64 accumulates the softmax denominator for free
  6. normalize via reciprocal + PE broadcast + DVE multiply (flash-style
     deferred normalization: applied to ctx, not to the 16.8M scores)
  7. out_partial[tok, 1024] = ctx (stationary) @ WoT slice
"""

import os
import numpy as np
import ml_dtypes

import concourse.bass as bass
import concourse.mybir as mybir
import concourse.tile as tile
from concourse import bacc
from concourse.bass_utils import run_bass_kernel_spmd

BF = mybir.dt.bfloat16
F32 = mybir.dt.float32
AF = mybir.ActivationFunctionType

NCORES = 8
B = 2
L = 2048
D = 1024          # d_model
H = 16            # heads
DK = 64           # head dim
HPC = H // NCORES  # heads per core = 2
PD = HPC * DK      # projection dims per core = 128
TOK = B * L        # 4096 tokens
P = 128

ROPE_BASE = 10000.0


def build_nc(debug_dumps=False):
    """Build the single-core Bass program (SPMD: same program, per-core data)."""
    from contextlib import ExitStack

    nc = bacc.Bacc("TRN2", target_bir_lowering=False, debug=False)
    dbg = {}
    if debug_dumps:
        for nm, shp, dt in [
            ("dbg_qq", [P, TOK], BF), ("dbg_kk", [P, TOK], BF),
            ("dbg_vh", [P, TOK], BF), ("dbg_vaug", [P, 64 * P], BF),
            ("dbg_exp", [P, 1024], BF), ("dbg_cp", [P, 1024], F32),
            ("dbg_rec", [2, 1024], F32), ("dbg_bcs", [P, 1024], F32),
            ("dbg_rsum", [2, 1024], F32),
            ("dbg_ctx", [P, TOK], BF),
        ]:
            dbg[nm] = nc.dram_tensor(nm, shp, dt, kind="ExternalOutput").ap()

    # ---- DRAM I/O ----
    qT = nc.dram_tensor("qT", [D, TOK], BF, kind="ExternalInput").ap()
    kT = nc.dram_tensor("kT", [D, TOK], BF, kind="ExternalInput").ap()
    vT = nc.dram_tensor("vT", [D, TOK], BF, kind="ExternalInput").ap()
    wqT = nc.dram_tensor("wqT", [D, PD], BF, kind="ExternalInput").ap()
    wkT = nc.dram_tensor("wkT", [D, PD], BF, kind="ExternalInput").ap()
    wvT = nc.dram_tensor("wvT", [D, PD], BF, kind="ExternalInput").ap()
    woT = nc.dram_tensor("woT", [PD, D], BF, kind="ExternalInput").ap()
    bq_d = nc.dram_tensor("bq", [PD, 1], F32, kind="ExternalInput").ap()
    bk_d = nc.dram_tensor("bk", [PD, 1], F32, kind="ExternalInput").ap()
    bv_d = nc.dram_tensor("bv", [PD, 1], F32, kind="ExternalInput").ap()
    cos_q = nc.dram_tensor("cos_q", [P, L], BF, kind="ExternalInput").ap()
    sin_q = nc.dram_tensor("sin_q", [P, L], BF, kind="ExternalInput").ap()
    cos_k = nc.dram_tensor("cos_k", [P, L], BF, kind="ExternalInput").ap()
    sin_k = nc.dram_tensor("sin_k", [P, L], BF, kind="ExternalInput").ap()
    outp = nc.dram_tensor("outp", [TOK, D], BF, kind="ExternalOutput").ap()

    with tile.TileContext(nc) as tc, ExitStack() as ctx:
        const = ctx.enter_context(tc.tile_pool(name="const", bufs=1))
        persist = ctx.enter_context(tc.tile_pool(name="persist", bufs=1))
        stage = ctx.enter_context(tc.tile_pool(name="stage", bufs=6))
        raws = ctx.enter_context(tc.tile_pool(name="raws", bufs=2))
        rots = ctx.enter_context(tc.tile_pool(name="rots", bufs=2))
        expp = ctx.enter_context(tc.tile_pool(name="expp", bufs=4))
        outs = ctx.enter_context(tc.tile_pool(name="outs", bufs=3))
        smalls = ctx.enter_context(tc.tile_pool(name="smalls", bufs=2))
        mmp = ctx.enter_context(tc.tile_pool(name="mmp", bufs=2, space="PSUM"))
        ctxp = ctx.enter_context(tc.tile_pool(name="ctxp", bufs=1, space="PSUM"))
        rsp = ctx.enter_context(tc.tile_pool(name="rsp", bufs=1, space="PSUM"))
        vhtp = ctx.enter_context(tc.tile_pool(name="vhtp", bufs=1))

        # ---- constants into SBUF (emitted in phase order so the first
        # projection's matmuls aren't queued behind 6MB of const DMA) ----
        def load_w(name, w_d):
            w_sb = const.tile([P, 8 * P], BF, name=name)
            nc.sync.dma_start(
                w_sb.rearrange("p (a m) -> p a m", a=8),
                w_d.rearrange("(a p) m -> p a m", p=P),
            )
            return w_sb

        def load_c(name, t_d, shape):
            t_sb = const.tile([P, shape], BF, name=name)
            nc.sync.dma_start(t_sb[:], t_d[:])
            return t_sb

        def load_b(name, b_d):
            b_sb = const.tile([P, 1], F32, name=name)
            nc.sync.dma_start(b_sb[:], b_d[:])
            return b_sb

        wq_sb = load_w("wq_sb", wqT)
        bq_sb = load_b("bq_sb", bq_d)
        cq_sb = load_c("cq_sb", cos_q, L)
        sq_sb = load_c("sq_sb", sin_q, L)

        # persistent activations
        qq_sb = persist.tile([P, TOK], BF)   # roped q-heads  [128 dims, 4096 tok]
        kk_sb = persist.tile([P, TOK], BF)   # roped k-heads
        vh_sb = persist.tile([P, TOK], BF)   # v-heads (dims-major)
        ctx_sb = persist.tile([P, TOK], BF)  # normalized attention ctx
        # Block-diagonal attention operands (both heads packed into K=128 so
        # the PE array runs fully occupied and the HAM clock-gate opens to
        # 2.4 GHz — K=64 matmul streams were measured to stay at 1.2 GHz):
        #   kh2[b]: 32 chunk tiles [128, 128]; chunk c is
        #           [[kh_h0[d, ktA] , 0], [0, kh_h1[d, ktA]]], ktA = 64 tokens
        #   vh2[b]: 32 chunk tiles [128, 128]; chunk c is
        #           [[vh_h0[ktA, d] , 0], [0, vh_h1[ktA, d]]]
        kh2 = [persist.tile([P, 32 * P], BF, name=f"kh2_{b}") for b in range(B)]
        vh2 = [persist.tile([P, 32 * P], BF, name=f"vh2_{b}") for b in range(B)]
        for t in kh2 + vh2:
            nc.gpsimd.memset(t[:], 0.0)
        # ones2: col 0 sums h0 rows (k 0..63), col 64 sums h1 rows; rest zero
        # (rowsums land on partitions 0 and 64 — legal AP base partitions).
        # Padded to a full 128-wide stationary so the rowsum matmuls keep the
        # PE array fully active.
        ones2 = const.tile([P, P], BF)
        nc.gpsimd.memset(ones2[:], 0.0)
        nc.vector.memset(ones2[0:DK, 0:1], 1.0)
        nc.vector.memset(ones2[DK:P, DK:DK + 1], 1.0)

        # ---------- phase helpers ----------
        def proj(x_d, w_sb, bias_sb, g, dst_sb, cos_sb=None, sin_sb=None):
            """Project token half g (2048 tokens) and optionally apply RoPE.

            Writes dst_sb[:, g*2048:(g+1)*2048] (bf16).
            """
            ps = [mmp.tile([P, 1024], F32, name=f"pj{g}_{half}", tag="mm")
                  for half in range(2)]
            for kt in range(8):
                xt = stage.tile([P, L], BF, name="xstage", tag="stage")
                nc.sync.dma_start(
                    xt[:], x_d[kt * P:(kt + 1) * P, g * L:(g + 1) * L])
                for half in range(2):
                    for nb in range(2):
                        c0 = half * 1024 + nb * 512
                        nc.tensor.matmul(
                            ps[half][:, nb * 512:(nb + 1) * 512],
                            lhsT=w_sb[:, kt * P:(kt + 1) * P],
                            rhs=xt[:, c0:c0 + 512],
                            start=(kt == 0), stop=(kt == 7),
                        )
            if cos_sb is None:
                # no rope (v): evict straight to destination
                for half in range(2):
                    nc.scalar.activation(
                        dst_sb[:, g * L + half * 1024: g * L + (half + 1) * 1024],
                        ps[half][:], AF.Identity, bias=bias_sb[:])
                return
            raw = raws.tile([P, L], BF, name="raw", tag="raw")
            for half in range(2):
                nc.scalar.activation(
                    raw[:, half * 1024:(half + 1) * 1024],
                    ps[half][:], AF.Identity, bias=bias_sb[:])
            rot = rots.tile([P, L], BF, name="rot", tag="rot")
            # rotate-half as partition-block moves (sign folded into sin table)
            for h in range(HPC):
                r0 = h * DK
                nc.sync.dma_start(rot[r0:r0 + 32, :], raw[r0 + 32:r0 + 64, :])
                nc.sync.dma_start(rot[r0 + 32:r0 + 64, :], raw[r0:r0 + 32, :])
            dst = dst_sb[:, g * L:(g + 1) * L]
            nc.vector.tensor_mul(raw[:], raw[:], cos_sb[:])
            nc.vector.tensor_mul(rot[:], rot[:], sin_sb[:])
            nc.vector.tensor_add(dst, raw[:], rot[:])

        def build_kv2(b):
            """Fill this batch's block-diagonal kh2/vh2 operand buffers."""
            # kh2: both copies are partition-aligned (h1 dims already live on
            # partitions 64..127 of kk_sb)
            kh2_r = kh2[b].rearrange("p (c u) -> p c u", u=P)
            kk_b = kk_sb[:, b * L:(b + 1) * L]
            nc.vector.tensor_copy(
                kh2_r[0:DK, :, 0:DK],
                kk_b[0:DK, :].rearrange("p (c u) -> p c u", u=DK))
            nc.vector.tensor_copy(
                kh2_r[DK:P, :, DK:P],
                kk_b[DK:P, :].rearrange("p (c u) -> p c u", u=DK))
            # vh2 needs [token, dim] tiles: PE-transpose 128-token tiles of
            # vh_sb, then 4 strided SBUF->SBUF DMAs place the 64-token
            # half-tiles into their diagonal blocks
            vht = vhtp.tile([P, 16 * P], BF, name="vht", tag="vht")
            for t in range(16):
                pt = mmp.tile([P, P], BF, name="pt", tag="mm")
                nc.tensor.transpose(
                    pt[:], vh_sb[:, b * L + t * P: b * L + (t + 1) * P],
                    ident[:])
                nc.vector.tensor_copy(vht[:, t * P:(t + 1) * P], pt[:])
            vht_r = vht.rearrange("p (t u) -> p t u", u=P)
            vh2_r = vh2[b].rearrange("p (t x) -> p t x", x=2 * P)
            # even chunks come from vht rows 0..63, odd chunks from 64..127
            nc.sync.dma_start(vh2_r[0:DK, :, 0:DK], vht_r[0:DK, :, 0:DK])
            nc.sync.dma_start(vh2_r[0:DK, :, 2 * DK:3 * DK], vht_r[DK:P, :, 0:DK])
            nc.sync.dma_start(vh2_r[DK:P, :, DK:2 * DK], vht_r[0:DK, :, DK:P])
            nc.sync.dma_start(vh2_r[DK:P, :, 3 * DK:4 * DK], vht_r[DK:P, :, DK:P])

        def attention(b, q2):
            """Both heads at once via block-diagonal K=128 matmuls.

            Scores chunk c: sc[0:64]  = scoresT_h0[ktA, qt],
                            sc[64:128] = scoresT_h1[ktA, qt].
            ctx2 accumulates [h0 dims | h1 dims, qt]; a parallel ones2 matmul
            accumulates both heads' softmax denominators in rows 0/1 of rs.
            Software-pipelined so the PE never waits on ScalarE's exp.
            """
            qs = qq_sb[:, b * L + q2 * 1024: b * L + (q2 + 1) * 1024]
            cp = ctxp.tile([P, 1024], F32, name="cp", tag="ctx")
            rs = rsp.tile([P, 1024], F32, name="rs", tag="rs")
            ex_prev = None
            for c in range(33):
                ex_cur = None
                if c < 32:
                    sc = mmp.tile([P, 1024], F32, name="sc", tag="mm")
                    for nb in range(2):
                        nc.tensor.matmul(
                            sc[:, nb * 512:(nb + 1) * 512],
                            lhsT=kh2[b][:, c * P:(c + 1) * P],
                            rhs=qs[:, nb * 512:(nb + 1) * 512],
                            start=True, stop=True, skip_group_check=True,
                        )
                    ex_cur = expp.tile([P, 1024], BF, name="ex", tag="exp")
                    nc.scalar.activation(ex_cur[:], sc[:], AF.Exp)
                    if debug_dumps and b == 0 and q2 == 0 and c == 0:
                        nc.sync.dma_start(dbg["dbg_exp"][:], ex_cur[:])
                if c >= 1:
                    cpv = c - 1
                    for nb in range(2):
                        sl = slice(nb * 512, (nb + 1) * 512)
                        nc.tensor.matmul(
                            cp[:, sl], lhsT=vh2[b][:, cpv * P:(cpv + 1) * P],
                            rhs=ex_prev[:, sl],
                            start=(cpv == 0), stop=(cpv == 31),
                            skip_group_check=True,
                        )
                        nc.tensor.matmul(
                            rs[:, sl], lhsT=ones2[:], rhs=ex_prev[:, sl],
                            start=(cpv == 0), stop=(cpv == 31),
                            skip_group_check=True,
                        )
                ex_prev = ex_cur
            # normalize: evict fast, then finish in SBUF off the PE path.
            # (base-64-partition custom DVE/GpSimd ops misbehave on HW, so
            # everything runs on partition-0-based tiles with small DMAs
            # doing the partition moves)
            rsum = smalls.tile([65, 1024], F32, name="rsum", tag="rsum")
            nc.vector.tensor_copy(rsum[:], rs[0:65, :])
            rsA = smalls.tile([1, 1024], F32, name="rsA", tag="rsA")
            rsB = smalls.tile([1, 1024], F32, name="rsB", tag="rsB")
            nc.sync.dma_start(rsA[:], rsum[0:1, :])
            nc.sync.dma_start(rsB[:], rsum[64:65, :])
            recA = smalls.tile([1, 1024], F32, name="recA", tag="recA")
            recB = smalls.tile([1, 1024], F32, name="recB", tag="recB")
            nc.vector.reciprocal_approx_fast(recA[:], rsA[:])
            nc.vector.reciprocal_approx_fast(recB[:], rsB[:])
            craw = smalls.tile([P, 1024], BF, name="craw", tag="craw")
            nc.vector.tensor_copy(craw[:], cp[:])
            bcs = smalls.tile([P, 1024], F32, name="bcs", tag="bcs")
            bcsB = smalls.tile([DK, 1024], F32, name="bcsB", tag="bcsB")
            nc.gpsimd.partition_broadcast(bcs[0:DK, :], recA[:], channels=DK)
            nc.gpsimd.partition_broadcast(bcsB[:], recB[:], channels=DK)
            nc.sync.dma_start(bcs[DK:P, :], bcsB[:])
            if debug_dumps and b == 0 and q2 == 0:
                cpd = smalls.tile([P, 1024], F32, name="cpd", tag="cpd", bufs=1)
                nc.vector.tensor_copy(cpd[:], cp[:])
                nc.sync.dma_start(dbg["dbg_cp"][:], cpd[:])
                nc.sync.dma_start(dbg["dbg_rec"][0:1, :], recA[:])
                nc.sync.dma_start(dbg["dbg_rec"][1:2, :], recB[:])
                nc.sync.dma_start(dbg["dbg_rsum"][0:1, :], rsum[0:1, :])
                nc.sync.dma_start(dbg["dbg_rsum"][1:2, :], rsum[64:65, :])
                nc.sync.dma_start(dbg["dbg_bcs"][:], bcs[:])
            c0 = b * L + q2 * 1024
            nc.vector.tensor_mul(ctx_sb[:, c0:c0 + 1024], craw[:], bcs[:])

        def out_proj(b, q2):
            for tb in range(q2 * 8, q2 * 8 + 8):
                t0 = b * L + tb * P
                po = mmp.tile([P, D], F32, name="po", tag="mm")
                for nb in range(2):
                    nc.tensor.matmul(
                        po[:, nb * 512:(nb + 1) * 512],
                        lhsT=ctx_sb[:, t0:t0 + P],
                        rhs=wo_sb[:, nb * 512:(nb + 1) * 512],
                        start=True, stop=True, skip_group_check=True,
                    )
                ob = outs.tile([P, D], BF, name="ob", tag="out")
                nc.vector.tensor_copy(ob[:], po[:])
                nc.sync.dma_start(outp[t0:t0 + P, :], ob[:])

        # ---------- program ----------
        proj(qT, wq_sb, bq_sb, 0, qq_sb, cq_sb, sq_sb)
        wk_sb = load_w("wk_sb", wkT)
        bk_sb = load_b("bk_sb", bk_d)
        ck_sb = load_c("ck_sb", cos_k, L)
        sk_sb = load_c("sk_sb", sin_k, L)
        proj(kT, wk_sb, bk_sb, 0, kk_sb, ck_sb, sk_sb)
        wv_sb = load_w("wv_sb", wvT)
        bv_sb = load_b("bv_sb", bv_d)
        ident = const.tile([P, P], BF)
        from concourse.masks import make_identity
        make_identity(nc, ident[:])
        wo_sb = const.tile([P, D], BF)
        nc.sync.dma_start(wo_sb[:], woT[:])
        proj(vT, wv_sb, bv_sb, 0, vh_sb)
        build_kv2(0)
        attention(0, 0)
        attention(0, 1)
        out_proj(0, 0)
        out_proj(0, 1)
        proj(qT, wq_sb, bq_sb, 1, qq_sb, cq_sb, sq_sb)
        proj(kT, wk_sb, bk_sb, 1, kk_sb, ck_sb, sk_sb)
        proj(vT, wv_sb, bv_sb, 1, vh_sb)
        build_kv2(1)
        attention(1, 0)
        out_proj(1, 0)
        attention(1, 1)
        out_proj(1, 1)

        if debug_dumps:
            nc.sync.dma_start(dbg["dbg_qq"][:], qq_sb[:])
            nc.sync.dma_start(dbg["dbg_kk"][:], kk_sb[:])
            nc.sync.dma_start(dbg["dbg_vh"][:], vh_sb[:])
            nc.sync.dma_start(dbg["dbg_ctx"][:], ctx_sb[:])
            nc.sync.dma_start(dbg["dbg_vaug"][:, 0:32 * P], kh2[0][:])
            nc.sync.dma_start(dbg["dbg_vaug"][:, 32 * P:64 * P], vh2[0][:])

    return nc


def _rope_tables():
    """Host-built RoPE tables, transposed to [d, t], 2 heads stacked.

    sin is sign-folded for the rotate-half convention; q tables carry the
    1/sqrt(dk) attention scale.
    """
    inv_freq = 1.0 / (ROPE_BASE ** (np.arange(0, DK, 2, dtype=np.float64) / DK))
    t = np.arange(L, dtype=np.float64)
    ang = np.outer(t, inv_freq)               # [L, 32]
    emb = np.concatenate([ang, ang], axis=1)  # [L, 64]
    cos = np.cos(emb).T.astype(np.float32)    # [64, L]
    sin = np.sin(emb).T.astype(np.float32)
    sin_folded = sin.copy()
    sin_folded[:32] *= -1.0
    scale = 1.0 / np.sqrt(DK)
    cos2 = np.concatenate([cos, cos], axis=0)                # [128, L]
    sin2 = np.concatenate([sin_folded, sin_folded], axis=0)  # [128, L]
    bf = ml_dtypes.bfloat16
    return (
        (cos2 * scale).astype(bf), (sin2 * scale).astype(bf),
        cos2.astype(bf), sin2.astype(bf),
    )


_NC_CACHE = {}


def _get_nc():
    if "nc" not in _NC_CACHE:
        nc = build_nc()
        nc.finalize()
        _NC_CACHE["nc"] = nc
    return _NC_CACHE["nc"]


def kernel(q, k, v, Wq, bq, Wk, bk, Wv, bv, Wo, bo):
    assert q.shape == (B, L, D) and k.shape == (B, L, D) and v.shape == (B, L, D)
    bf = ml_dtypes.bfloat16
    qT = np.ascontiguousarray(q.reshape(TOK, D).T).astype(bf)
    kT = np.ascontiguousarray(k.reshape(TOK, D).T).astype(bf)
    vT = np.ascontiguousarray(v.reshape(TOK, D).T).astype(bf)
    cos_q, sin_q, cos_k, sin_k = _rope_tables()

    in_maps = []
    for c in range(NCORES):
        hs = slice(c * PD, (c + 1) * PD)
        in_maps.append({
            "qT": qT, "kT": kT, "vT": vT,
            "wqT": np.ascontiguousarray(Wq[hs, :].T).astype(bf),
            "wkT": np.ascontiguousarray(Wk[hs, :].T).astype(bf),
            "wvT": np.ascontiguousarray(Wv[hs, :].T).astype(bf),
            "woT": np.ascontiguousarray(Wo[:, hs].T).astype(bf),
            "bq": np.asarray(bq[hs], np.float32).reshape(PD, 1),
            "bk": np.asarray(bk[hs], np.float32).reshape(PD, 1),
            "bv": np.asarray(bv[hs], np.float32).reshape(PD, 1),
            "cos_q": cos_q, "sin_q": sin_q, "cos_k": cos_k, "sin_k": sin_k,
        })

    nc = _get_nc()
    res = run_bass_kernel_spmd(nc, in_maps, list(range(NCORES)))
    out = np.zeros((TOK, D), np.float64)
    for r in res.results:
        out += r["outp"].astype(np.float64)
    out += np.asarray(bo, np.float64)[None, :]
    return out.astype(np.float32).reshape(B, L, D)



# revision 8
# speedup vs baseline: 1.3597x; 1.3597x over previous
"""Trainium2 Bass kernel for MultiHeadAttention with RoPE.

Problem: B=2, L=2048, d_model=1024, 16 heads, d_k=64, fp32 in/out.

Sharding (8 cores): tensor-parallel over heads — core c owns heads
{2c, 2c+1}, i.e. a 128-wide slice of the projection output dims.  Every
core reads the full q/k/v activations (transposed + bf16 on host), its
own 128-row slice of Wq/Wk/Wv (pre-transposed; Wq/bq pre-scaled by
1/sqrt(dk)) and the matching 128 columns of Wo.  Each core computes its
heads' attention output and a partial d_model output projection; the
host sums the 8 partials and adds bo.

Per-core pipeline (bf16 matmuls, fp32 PSUM):
  1. QKV projections [128 pd, 1024 tok] halves; bias-add + bf16 evict on
     DVE; RoPE via partition-swap DMAs + 3 DVE ops (sign folded in sin
     table, 1/sqrt(dk) folded into Wq).
  2. V-heads transposed to [kt, dim] layout by ONE dma_start_transpose
     per (batch, head) into a 65-wide-stride "vaug" buffer whose 65th
     column is ones.
  3. Scores: per 128-kt tile, the two heads run CONCURRENTLY on the PE
     as K=64 row-tiles (tile_position (0,0) / (64,0) auto-derived).
  4. exp on ScalarE ([128, 1024] per kt tile covering both heads).
  5. ctx: lhsT = vaug [128 kt, 65] per head; row 64 accumulates the
     softmax denominator for free (M=65 stationary).
  6. normalize: DVE copy of cp, denominator row -> partition 0 via DMA,
     reciprocal + GpSimd broadcast + DVE muls; h1 ctx shifted to
     partitions 64:127 by a small DMA.
  7. out_proj [tok, 1024] = ctx (stationary) @ WoT slice.
Emission interleaves next-batch projections and out_proj matmuls into
the (ScalarE-bound) attention loops as "fillers" so the PE never idles
long and HAM stays at full clock.  All activation buffers are
per-batch tiles so filler writes never create false WAR dependencies
against the running attention.
"""

import collections
import numpy as np
import ml_dtypes

import concourse.bass as bass
import concourse.mybir as mybir
import concourse.tile as tile
from concourse import bacc
from concourse.bass_utils import run_bass_kernel_spmd

BF = mybir.dt.bfloat16
F32 = mybir.dt.float32
AF = mybir.ActivationFunctionType

NCORES = 8
B = 2
L = 2048
D = 1024          # d_model
H = 16            # heads
DK = 64           # head dim
HPC = H // NCORES  # heads per core = 2
PD = HPC * DK      # projection dims per core = 128
TOK = B * L        # 4096 tokens
P = 128
NKT = L // P       # 16 kt tiles per batch
NQB = 4            # 512-token q blocks per batch

ROPE_BASE = 10000.0


def build_nc(debug_dumps=False):
    """Build the single-core Bass program (SPMD: same program, per-core data)."""
    from contextlib import ExitStack

    nc = bacc.Bacc("TRN2", target_bir_lowering=False, debug=False)
    dbg = {}
    if debug_dumps:
        for nm, shp, dt in [
            ("dbg_qq", [P, L], BF), ("dbg_kk", [P, L], BF),
            ("dbg_vaug0", [P, NKT * 65], BF), ("dbg_vaug1", [P, NKT * 65], BF),
            ("dbg_exp", [P, 1024], BF), ("dbg_cps", [65, 1024], F32),
            ("dbg_rec", [1, 1024], F32), ("dbg_ctx", [P, L], BF),
        ]:
            dbg[nm] = nc.dram_tensor(nm, shp, dt, kind="ExternalOutput").ap()

    # ---- DRAM I/O ----
    qT = nc.dram_tensor("qT", [D, TOK], BF, kind="ExternalInput").ap()
    kT = nc.dram_tensor("kT", [D, TOK], BF, kind="ExternalInput").ap()
    vT = nc.dram_tensor("vT", [D, TOK], BF, kind="ExternalInput").ap()
    wqT = nc.dram_tensor("wqT", [D, PD], BF, kind="ExternalInput").ap()
    wkT = nc.dram_tensor("wkT", [D, PD], BF, kind="ExternalInput").ap()
    wvT = nc.dram_tensor("wvT", [D, PD], BF, kind="ExternalInput").ap()
    woT = nc.dram_tensor("woT", [PD, D], BF, kind="ExternalInput").ap()
    bq_d = nc.dram_tensor("bq", [PD, 1], F32, kind="ExternalInput").ap()
    bk_d = nc.dram_tensor("bk", [PD, 1], F32, kind="ExternalInput").ap()
    bv_d = nc.dram_tensor("bv", [PD, 1], F32, kind="ExternalInput").ap()
    cos_d = nc.dram_tensor("cos_t", [P, L], BF, kind="ExternalInput").ap()
    sin_d = nc.dram_tensor("sin_t", [P, L], BF, kind="ExternalInput").ap()
    outp = nc.dram_tensor("outp", [TOK, D], BF, kind="ExternalOutput").ap()

    xT = {"q": qT, "k": kT, "v": vT}

    with tile.TileContext(nc) as tc, ExitStack() as ctx:
        const = ctx.enter_context(tc.tile_pool(name="const", bufs=1))
        persist = ctx.enter_context(tc.tile_pool(name="persist", bufs=1))
        stage = ctx.enter_context(tc.tile_pool(name="stage", bufs=8))
        raws = ctx.enter_context(tc.tile_pool(name="raws", bufs=2))
        rots = ctx.enter_context(tc.tile_pool(name="rots", bufs=2))
        expp = ctx.enter_context(tc.tile_pool(name="expp", bufs=3))
        outs = ctx.enter_context(tc.tile_pool(name="outs", bufs=3))
        smalls = ctx.enter_context(tc.tile_pool(name="smalls", bufs=2))
        h1p = ctx.enter_context(tc.tile_pool(name="h1p", bufs=2))
        # PSUM: scores 2 tiles x 2 banks + ctx 2 banks + proj/out 2 banks = 8
        scp = ctx.enter_context(tc.tile_pool(name="scp", bufs=2, space="PSUM"))
        cpp = ctx.enter_context(tc.tile_pool(name="cpp", bufs=1, space="PSUM"))
        pop = ctx.enter_context(tc.tile_pool(name="pop", bufs=1, space="PSUM"))

        # ---- constants (emitted in first-use order) ----
        def load_w(name, w_d):
            w_sb = const.tile([P, 8 * P], BF, name=name)
            nc.sync.dma_start(
                w_sb.rearrange("p (a m) -> p a m", a=8),
                w_d.rearrange("(a p) m -> p a m", p=P),
            )
            return w_sb

        def load_b(name, b_d):
            b_sb = const.tile([P, 1], F32, name=name)
            nc.sync.dma_start(b_sb[:], b_d[:])
            return b_sb

        wk_sb = load_w("wk_sb", wkT)
        bk_sb = load_b("bk_sb", bk_d)
        cos_sb = const.tile([P, L], BF)
        nc.sync.dma_start(cos_sb[:], cos_d[:])
        sin_sb = const.tile([P, L], BF)
        nc.sync.dma_start(sin_sb[:], sin_d[:])

        # per-batch persistent activations [128 dims, 2048 tok]
        qq_b = [persist.tile([P, L], BF, name=f"qq{b}") for b in range(B)]
        kk_b = [persist.tile([P, L], BF, name=f"kk{b}") for b in range(B)]
        ctx_b = [persist.tile([P, L], BF, name=f"ctx{b}") for b in range(B)]
        # vaug[b][h]: 16 slots of [128 kt, 65]; cols 0:64 = v dims
        # (t-major: slot t partition p holds token t*128+p), col 64 = ones.
        vaug = [[persist.tile([P, NKT * 65], BF, name=f"vaug_{b}_{h}")
                 for h in range(2)] for b in range(B)]
        for b in range(B):
            for h in range(2):
                va = vaug[b][h].rearrange("p (t u) -> p t u", u=65)
                nc.vector.memset(va[:, :, 64:65], 1.0)

        # ---------- filler machinery ----------
        fillers = collections.deque()

        def fill(budget):
            while fillers and budget > 0:
                cost, fn = fillers.popleft()
                fn()
                budget -= cost

        def flush():
            while fillers:
                fillers.popleft()[1]()

        # ---------- phase helpers ----------
        def load_half(which, b, half):
            """4 kc-pair DMAs for a 1024-token half -> 4 stage tiles."""
            tiles = []
            src = xT[which].rearrange("(a p) t -> p a t", p=P)
            for cp2 in range(4):
                xt = stage.tile([P, 2 * 1024], BF, name="xstage", tag="stage")
                nc.sync.dma_start(
                    xt.rearrange("p (a t) -> p a t", a=2),
                    src[:, 2 * cp2:2 * cp2 + 2,
                        b * L + half * 1024: b * L + (half + 1) * 1024],
                )
                tiles.append(xt)
            return tiles

        def proj_units(which, b, w_sb, bias_sb, dst_sb=None, vh_cb=None):
            """Filler units projecting batch b (2 halves of 1024 tokens).

            dst_sb given -> rope into it (q/k).  vh_cb given -> v path:
            evict to a fresh vh tile, call vh_cb(vh_tile) when done.
            """
            units = []
            shared = {}

            def load(half):
                def go():
                    shared[("x", half)] = load_half(which, b, half)
                return go

            def alloc_pp(half):
                def go():
                    shared[("pp", half)] = pop.tile(
                        [P, 1024], F32, name="pp", tag="pp")
                return go

            def mm_kc(half, kc):
                def go():
                    xt = shared[("x", half)][kc // 2].rearrange(
                        "p (a t) -> p a t", a=2)
                    pp = shared[("pp", half)]
                    for nb in range(2):
                        nc.tensor.matmul(
                            pp[:, nb * 512:(nb + 1) * 512],
                            lhsT=w_sb[:, kc * P:(kc + 1) * P],
                            rhs=xt[:, kc % 2, nb * 512:(nb + 1) * 512],
                            start=(kc == 0), stop=(kc == 7),
                        )
                return go

            def evict_rope(half):
                def go():
                    pp = shared[("pp", half)]
                    raw = raws.tile([P, 1024], BF, name="raw", tag="raw")
                    nc.vector.tensor_scalar_add(raw[:], pp[:], bias_sb[:])
                    rot = rots.tile([P, 1024], BF, name="rot", tag="rot")
                    for h in range(2):
                        r0 = h * DK
                        nc.sync.dma_start(rot[r0:r0 + 32, :],
                                          raw[r0 + 32:r0 + 64, :])
                        nc.sync.dma_start(rot[r0 + 32:r0 + 64, :],
                                          raw[r0:r0 + 32, :])
                    cs = slice(half * 1024, (half + 1) * 1024)
                    nc.vector.tensor_mul(raw[:], raw[:], cos_sb[:, cs])
                    nc.vector.tensor_mul(rot[:], rot[:], sin_sb[:, cs])
                    nc.vector.tensor_add(dst_sb[:, cs], raw[:], rot[:])
                return go

            def evict_v(half):
                def go():
                    pp = shared[("pp", half)]
                    if "vh" not in shared:
                        shared["vh"] = raws.tile(
                            [P, L], BF, name="vhs", tag="vraw", bufs=2)
                    vh = shared["vh"]
                    nc.vector.tensor_scalar_add(
                        vh[:, half * 1024:(half + 1) * 1024], pp[:], bias_sb[:])
                    if half == 1:
                        vh_cb(vh)
                return go

            units.append((4, load(0)))
            units.append((4, load(1)))
            for half in range(2):
                units.append((0, alloc_pp(half)))
                for kc in range(8):
                    units.append((2, mm_kc(half, kc)))
                units.append((0, evict_rope(half) if dst_sb is not None
                              else evict_v(half)))
            return units

        def vaug_transpose(b):
            # HW xbar transpose requires a contiguous destination; land in
            # vt then DVE-copy into the 65-stride vaug slots.
            def go(vh):
                vt = rots.tile([P, L], BF, name="vt", tag="vt", bufs=2)
                vt_r = vt.rearrange("p (h t u) -> p h t u", h=2, u=64)
                for h in range(2):
                    nc.sync.dma_start_transpose(
                        vt_r[:, h], vh[h * DK:(h + 1) * DK, :])
                for h in range(2):
                    va = vaug[b][h].rearrange(
                        "p (t u) -> p t u", u=65)[:, :, 0:64]
                    nc.vector.tensor_copy(va, vt_r[:, h])
            return go

        def attention(b, qb):
            """512 q tokens; 16 kt tiles; 2 heads row-tiled on the PE."""
            q0 = qb * 512
            cp = cpp.tile([65, 1024], F32, name="cp", tag="cp")
            ex_prev = None
            for kt in range(NKT + 1):
                ex_cur = None
                if kt < NKT:
                    k0 = kt * P
                    sc = scp.tile([P, 1024], F32, name="sc", tag="sc")
                    nc.tensor.matmul(
                        sc[:, 0:512],
                        lhsT=kk_b[b][0:DK, k0:k0 + P],
                        rhs=qq_b[b][0:DK, q0:q0 + 512],
                        start=True, stop=True, skip_group_check=True)
                    nc.tensor.matmul(
                        sc[:, 512:1024],
                        lhsT=kk_b[b][DK:P, k0:k0 + P],
                        rhs=qq_b[b][DK:P, q0:q0 + 512],
                        start=True, stop=True, skip_group_check=True)
                    ex_cur = expp.tile([P, 1024], BF, name="ex", tag="ex")
                    nc.scalar.activation(ex_cur[:], sc[:], AF.Exp)
                    if debug_dumps and b == 0 and qb == 0 and kt == 0:
                        nc.sync.dma_start(dbg["dbg_exp"][:], ex_cur[:])
                if kt >= 1:
                    c = kt - 1
                    for h in range(2):
                        nc.tensor.matmul(
                            cp[:, h * 512:(h + 1) * 512],
                            lhsT=vaug[b][h][:, c * 65:(c + 1) * 65],
                            rhs=ex_prev[:, h * 512:(h + 1) * 512],
                            start=(c == 0), stop=(c == NKT - 1),
                            skip_group_check=True)
                ex_prev = ex_cur
                fill(2)
            # normalize: copy cp out (frees the psum), move den row to
            # partition 0, reciprocal, broadcast, scale both heads.
            cps = smalls.tile([65, 1024], F32, name="cps", tag="cps")
            nc.vector.tensor_copy(cps[:], cp[:])
            den = smalls.tile([1, 1024], F32, name="den", tag="den")
            nc.sync.dma_start(den[:], cps[64:65, :])
            rec = smalls.tile([1, 1024], F32, name="rec", tag="rec")
            nc.vector.reciprocal_approx_fast(rec[:], den[:])
            if debug_dumps and b == 0 and qb == 0:
                nc.sync.dma_start(dbg["dbg_cps"][:], cps[:])
                nc.sync.dma_start(dbg["dbg_rec"][:], rec[:])
            bcs = smalls.tile([DK, 1024], F32, name="bcs", tag="bcs")
            nc.gpsimd.partition_broadcast(bcs[:], rec[:], channels=DK)
            nc.vector.tensor_mul(
                ctx_b[b][0:DK, q0:q0 + 512], cps[0:DK, 0:512], bcs[:, 0:512])
            h1s = h1p.tile([DK, 512], BF, name="h1s", tag="h1s")
            nc.vector.tensor_mul(h1s[:], cps[0:DK, 512:1024], bcs[:, 512:1024])
            nc.sync.dma_start(ctx_b[b][DK:P, q0:q0 + 512], h1s[:])

        def out_units(b, qb):
            units = []

            def po_unit(tb):
                def go():
                    t0 = qb * 512 + tb * P
                    po = pop.tile([P, D], F32, name="po", tag="pp")
                    for nb in range(2):
                        nc.tensor.matmul(
                            po[:, nb * 512:(nb + 1) * 512],
                            lhsT=ctx_b[b][:, t0:t0 + P],
                            rhs=wo_sb[:, nb * 512:(nb + 1) * 512],
                            start=True, stop=True, skip_group_check=True)
                    ob = outs.tile([P, D], BF, name="ob", tag="ob")
                    nc.vector.tensor_copy(ob[:], po[:])
                    nc.sync.dma_start(outp[b * L + t0:b * L + t0 + P, :], ob[:])
                return go

            for tb in range(4):
                units.append((2, po_unit(tb)))
            return units

        # ---------- program ----------
        # batch 0 projections run dense (nothing to overlap with yet)
        for _, fn in proj_units("k", 0, wk_sb, bk_sb, dst_sb=kk_b[0]):
            fn()
        wv_sb = load_w("wv_sb", wvT)
        bv_sb = load_b("bv_sb", bv_d)
        for _, fn in proj_units("v", 0, wv_sb, bv_sb, vh_cb=vaug_transpose(0)):
            fn()
        wq_sb = load_w("wq_sb", wqT)
        bq_sb = load_b("bq_sb", bq_d)
        wo_sb = const.tile([P, D], BF)
        nc.sync.dma_start(wo_sb[:], woT[:])
        for _, fn in proj_units("q", 0, wq_sb, bq_sb, dst_sb=qq_b[0]):
            fn()

        # batch 0 attention; feed batch-1 projections + batch-0 out_proj
        # into the scalar-bound loop as fillers.
        for qb in range(NQB):
            attention(0, qb)
            fillers.extend(out_units(0, qb))
            if qb == 0:
                fillers.extend(proj_units("k", 1, wk_sb, bk_sb, dst_sb=kk_b[1]))
            elif qb == 1:
                fillers.extend(proj_units("v", 1, wv_sb, bv_sb,
                                          vh_cb=vaug_transpose(1)))
            elif qb == 2:
                fillers.extend(proj_units("q", 1, wq_sb, bq_sb, dst_sb=qq_b[1]))
        flush()  # b1 attention depends on b1 projections: drain first

        for qb in range(NQB):
            attention(1, qb)
            fillers.extend(out_units(1, qb))
        flush()

        if debug_dumps:
            nc.sync.dma_start(dbg["dbg_qq"][:], qq_b[0][:])
            nc.sync.dma_start(dbg["dbg_kk"][:], kk_b[0][:])
            nc.sync.dma_start(dbg["dbg_vaug0"][:], vaug[0][0][:])
            nc.sync.dma_start(dbg["dbg_vaug1"][:], vaug[0][1][:])
            nc.sync.dma_start(dbg["dbg_ctx"][:], ctx_b[0][:])

    return nc


def _rope_tables():
    """Host-built RoPE tables [d, t], 2 heads stacked, sign-folded sin."""
    inv_freq = 1.0 / (ROPE_BASE ** (np.arange(0, DK, 2, dtype=np.float64) / DK))
    t = np.arange(L, dtype=np.float64)
    ang = np.outer(t, inv_freq)               # [L, 32]
    emb = np.concatenate([ang, ang], axis=1)  # [L, 64]
    cos = np.cos(emb).T.astype(np.float32)    # [64, L]
    sin = np.sin(emb).T.astype(np.float32)
    sin_folded = sin.copy()
    sin_folded[:32] *= -1.0
    bf = ml_dtypes.bfloat16
    cos2 = np.concatenate([cos, cos], axis=0)                # [128, L]
    sin2 = np.concatenate([sin_folded, sin_folded], axis=0)  # [128, L]
    return cos2.astype(bf), sin2.astype(bf)


def host_in_maps(q, k, v, Wq, bq, Wk, bk, Wv, bv, Wo):
    """Per-core input maps (the 1/sqrt(dk) scale is folded into Wq/bq)."""
    bf = ml_dtypes.bfloat16
    qT = np.ascontiguousarray(np.asarray(q).reshape(TOK, D).T).astype(bf)
    kT = np.ascontiguousarray(np.asarray(k).reshape(TOK, D).T).astype(bf)
    vT = np.ascontiguousarray(np.asarray(v).reshape(TOK, D).T).astype(bf)
    cos_t, sin_t = _rope_tables()
    scale = 1.0 / np.sqrt(DK)
    in_maps = []
    for c in range(NCORES):
        hs = slice(c * PD, (c + 1) * PD)
        in_maps.append({
            "qT": qT, "kT": kT, "vT": vT,
            "wqT": np.ascontiguousarray(
                (np.asarray(Wq)[hs, :] * scale).T).astype(bf),
            "wkT": np.ascontiguousarray(np.asarray(Wk)[hs, :].T).astype(bf),
            "wvT": np.ascontiguousarray(np.asarray(Wv)[hs, :].T).astype(bf),
            "woT": np.ascontiguousarray(np.asarray(Wo)[:, hs].T).astype(bf),
            "bq": (np.asarray(bq)[hs] * scale).astype(np.float32).reshape(PD, 1),
            "bk": np.asarray(bk[hs], np.float32).reshape(PD, 1),
            "bv": np.asarray(bv[hs], np.float32).reshape(PD, 1),
            "cos_t": cos_t, "sin_t": sin_t,
        })
    return in_maps


_NC_CACHE = {}


def _get_nc():
    if "nc" not in _NC_CACHE:
        nc = build_nc()
        nc.finalize()
        _NC_CACHE["nc"] = nc
    return _NC_CACHE["nc"]


def kernel(q, k, v, Wq, bq, Wk, bk, Wv, bv, Wo, bo):
    assert q.shape == (B, L, D) and k.shape == (B, L, D) and v.shape == (B, L, D)
    in_maps = host_in_maps(q, k, v, Wq, bq, Wk, bk, Wv, bv, Wo)
    nc = _get_nc()
    res = run_bass_kernel_spmd(nc, in_maps, list(range(NCORES)))
    out = np.zeros((TOK, D), np.float64)
    for r in res.results:
        out += r["outp"].astype(np.float64)
    out += np.asarray(bo, np.float64)[None, :]
    return out.astype(np.float32).reshape(B, L, D)


# revision 16
# speedup vs baseline: 1.4276x; 1.0500x over previous
"""Trainium2 Bass kernel for MultiHeadAttention with RoPE.

Problem: B=2, L=2048, d_model=1024, 16 heads, d_k=64, fp32 in/out.

Sharding (8 cores): tensor-parallel over heads — core c owns heads
{2c, 2c+1}, i.e. a 128-wide slice of the projection output dims.  Every
core reads the full q/k/v activations (transposed + bf16 on host), its
own 128-row slice of Wq/Wk/Wv (pre-transposed; Wq/bq pre-scaled by
1/sqrt(dk)) and the matching 128 columns of Wo.  Each core computes its
heads' attention output and a partial d_model output projection; the
host sums the 8 partials and adds bo.

Per-core pipeline (bf16 matmuls, fp32 PSUM):
  1. QKV projections [128 pd, 1024 tok] halves; bias-add + bf16 evict on
     DVE; RoPE via partition-swap DMAs + 3 DVE ops (sign folded in sin
     table, 1/sqrt(dk) folded into Wq).
  2. V-heads transposed to [kt, dim] layout by ONE dma_start_transpose
     per (batch, head) into a 65-wide-stride "vaug" buffer whose 65th
     column is ones.
  3. Scores: per 128-kt tile, the two heads run CONCURRENTLY on the PE
     as K=64 row-tiles (tile_position (0,0) / (64,0) auto-derived).
  4. exp on ScalarE ([128, 1024] per kt tile covering both heads).
  5. ctx: lhsT = vaug [128 kt, 65] per head; row 64 accumulates the
     softmax denominator for free (M=65 stationary).
  6. normalize: DVE copy of cp, denominator row -> partition 0 via DMA,
     reciprocal + GpSimd broadcast + DVE muls; h1 ctx shifted to
     partitions 64:127 by a small DMA.
  7. out_proj [tok, 1024] = ctx (stationary) @ WoT slice.
Emission interleaves next-batch projections and out_proj matmuls into
the (ScalarE-bound) attention loops as "fillers" so the PE never idles
long and HAM stays at full clock.  All activation buffers are
per-batch tiles so filler writes never create false WAR dependencies
against the running attention.
"""

import collections
import numpy as np
import ml_dtypes

import concourse.bass as bass
import concourse.mybir as mybir
import concourse.tile as tile
from concourse import bacc
from concourse.bass_utils import run_bass_kernel_spmd

BF = mybir.dt.bfloat16
F32 = mybir.dt.float32
AF = mybir.ActivationFunctionType

NCORES = 8
B = 2
L = 2048
D = 1024          # d_model
H = 16            # heads
DK = 64           # head dim
HPC = H // NCORES  # heads per core = 2
PD = HPC * DK      # projection dims per core = 128
TOK = B * L        # 4096 tokens
P = 128
NKT = L // P       # 16 kt tiles per batch
NQB = 4            # 512-token q blocks per batch

ROPE_BASE = 10000.0


def build_nc(debug_dumps=False):
    """Build the single-core Bass program (SPMD: same program, per-core data)."""
    from contextlib import ExitStack

    nc = bacc.Bacc("TRN2", target_bir_lowering=False, debug=False)
    dbg = {}
    if debug_dumps:
        for nm, shp, dt in [
            ("dbg_qq", [P, L], BF), ("dbg_kk", [P, L], BF),
            ("dbg_vaug0", [P, NKT * 65], BF), ("dbg_vaug1", [P, NKT * 65], BF),
            ("dbg_exp", [P, 1024], BF), ("dbg_cps", [65, 1024], F32),
            ("dbg_rec", [1, 1024], F32), ("dbg_ctx", [P, L], BF),
        ]:
            dbg[nm] = nc.dram_tensor(nm, shp, dt, kind="ExternalOutput").ap()

    # ---- DRAM I/O ----
    qT = nc.dram_tensor("qT", [D, TOK], BF, kind="ExternalInput").ap()
    kT = nc.dram_tensor("kT", [D, TOK], BF, kind="ExternalInput").ap()
    vT = nc.dram_tensor("vT", [D, TOK], BF, kind="ExternalInput").ap()
    wqT = nc.dram_tensor("wqT", [D, PD], BF, kind="ExternalInput").ap()
    wkT = nc.dram_tensor("wkT", [D, PD], BF, kind="ExternalInput").ap()
    wvT = nc.dram_tensor("wvT", [D, PD], BF, kind="ExternalInput").ap()
    woT = nc.dram_tensor("woT", [PD, D], BF, kind="ExternalInput").ap()
    bq_d = nc.dram_tensor("bq", [PD, 1], F32, kind="ExternalInput").ap()
    bk_d = nc.dram_tensor("bk", [PD, 1], F32, kind="ExternalInput").ap()
    bv_d = nc.dram_tensor("bv", [PD, 1], F32, kind="ExternalInput").ap()
    cos_d = nc.dram_tensor("cos_t", [P, L], BF, kind="ExternalInput").ap()
    sin_d = nc.dram_tensor("sin_t", [P, L], BF, kind="ExternalInput").ap()
    outp = nc.dram_tensor("outp", [TOK, D], BF, kind="ExternalOutput").ap()

    xT = {"q": qT, "k": kT, "v": vT}

    with tile.TileContext(nc) as tc, ExitStack() as ctx:
        const = ctx.enter_context(tc.tile_pool(name="const", bufs=1))
        persist = ctx.enter_context(tc.tile_pool(name="persist", bufs=1))
        stage = ctx.enter_context(tc.tile_pool(name="stage", bufs=5))
        raws = ctx.enter_context(tc.tile_pool(name="raws", bufs=2))
        rots = ctx.enter_context(tc.tile_pool(name="rots", bufs=2))
        expp = ctx.enter_context(tc.tile_pool(name="expp", bufs=3))
        outs = ctx.enter_context(tc.tile_pool(name="outs", bufs=3))
        smalls = ctx.enter_context(tc.tile_pool(name="smalls", bufs=2))
        h1p = ctx.enter_context(tc.tile_pool(name="h1p", bufs=2))
        # PSUM: scores 2 tiles x 2 banks + ctx 2 banks + proj/out 2 banks = 8
        scp = ctx.enter_context(tc.tile_pool(name="scp", bufs=2, space="PSUM"))
        cpp = ctx.enter_context(tc.tile_pool(name="cpp", bufs=1, space="PSUM"))
        pop = ctx.enter_context(tc.tile_pool(name="pop", bufs=1, space="PSUM"))

        # ---- constants (emitted in first-use order) ----
        def load_w(name, w_d):
            w_sb = const.tile([P, 8 * P], BF, name=name)
            nc.sync.dma_start(
                w_sb.rearrange("p (a m) -> p a m", a=8),
                w_d.rearrange("(a p) m -> p a m", p=P),
            )
            return w_sb

        def load_b(name, b_d):
            b_sb = const.tile([P, 1], F32, name=name)
            nc.sync.dma_start(b_sb[:], b_d[:])
            return b_sb

        wk_sb = load_w("wk_sb", wkT)
        bk_sb = load_b("bk_sb", bk_d)
        cos_sb = const.tile([P, L], BF)
        nc.sync.dma_start(cos_sb[:], cos_d[:])
        sin_sb = const.tile([P, L], BF)
        nc.sync.dma_start(sin_sb[:], sin_d[:])

        # per-batch persistent activations [128 dims, 2048 tok]
        qq_b = [persist.tile([P, L], BF, name=f"qq{b}") for b in range(B)]
        kk_b = [persist.tile([P, L], BF, name=f"kk{b}") for b in range(B)]
        ctx_b = [persist.tile([P, L], BF, name=f"ctx{b}") for b in range(B)]
        # vaug[b][h]: 16 slots of [128 kt, 65]; cols 0:64 = v dims
        # (t-major: slot t partition p holds token t*128+p), col 64 = ones.
        vaug = [[persist.tile([P, NKT * 65], BF, name=f"vaug_{b}_{h}")
                 for h in range(2)] for b in range(B)]
        for b in range(B):
            for h in range(2):
                va = vaug[b][h].rearrange("p (t u) -> p t u", u=65)
                nc.vector.memset(va[:, :, 64:65], 1.0)

        # ---------- filler machinery ----------
        fillers = collections.deque()

        def fill(budget):
            while fillers and budget > 0:
                cost, fn = fillers.popleft()
                fn()
                budget -= cost

        def flush():
            while fillers:
                fillers.popleft()[1]()

        # ---------- phase helpers ----------
        def load_half(which, b, half):
            """One 2MB DMA: all 8 k-chunks of a 1024-token half."""
            xt = stage.tile([P, 8 * 1024], BF, name="xstage", tag="stage")
            nc.sync.dma_start(
                xt.rearrange("p (a t) -> p a t", a=8),
                xT[which].rearrange("(a p) t -> p a t", p=P)[
                    :, :, b * L + half * 1024: b * L + (half + 1) * 1024],
            )
            return xt

        def proj_units(which, b, w_sb, bias_sb, dst_sb=None, vh_cb=None,
                       preloaded=None):
            """Filler units projecting batch b (2 halves of 1024 tokens).

            dst_sb given -> rope into it (q/k).  vh_cb given -> v path:
            evict to a fresh vh tile, call vh_cb(vh_tile) when done.
            preloaded: stage tiles already loaded (batch-0 prefetch).
            """
            units = []
            shared = {}
            if preloaded is not None:
                shared[("x", 0)], shared[("x", 1)] = preloaded

            def load(half):
                def go():
                    shared[("x", half)] = load_half(which, b, half)
                return go

            def alloc_pp(half):
                def go():
                    shared[("pp", half)] = pop.tile(
                        [P, 1024], F32, name="pp", tag="pp")
                return go

            def mm_kc(half, kc):
                def go():
                    xt = shared[("x", half)].rearrange("p (a t) -> p a t", a=8)
                    pp = shared[("pp", half)]
                    for nb in range(2):
                        nc.tensor.matmul(
                            pp[:, nb * 512:(nb + 1) * 512],
                            lhsT=w_sb[:, kc * P:(kc + 1) * P],
                            rhs=xt[:, kc, nb * 512:(nb + 1) * 512],
                            start=(kc == 0), stop=(kc == 7),
                        )
                return go

            def evict_rope(half):
                def go():
                    pp = shared[("pp", half)]
                    raw = raws.tile([P, 1024], BF, name="raw", tag="raw")
                    nc.vector.tensor_scalar_add(raw[:], pp[:], bias_sb[:])
                    rot = rots.tile([P, 1024], BF, name="rot", tag="rot")
                    for h in range(2):
                        r0 = h * DK
                        nc.sync.dma_start(rot[r0:r0 + 32, :],
                                          raw[r0 + 32:r0 + 64, :])
                        nc.sync.dma_start(rot[r0 + 32:r0 + 64, :],
                                          raw[r0:r0 + 32, :])
                    cs = slice(half * 1024, (half + 1) * 1024)
                    nc.vector.tensor_mul(raw[:], raw[:], cos_sb[:, cs])
                    nc.vector.tensor_mul(rot[:], rot[:], sin_sb[:, cs])
                    nc.vector.tensor_add(dst_sb[:, cs], raw[:], rot[:])
                return go

            def evict_v(half):
                def go():
                    pp = shared[("pp", half)]
                    if "vh" not in shared:
                        shared["vh"] = raws.tile(
                            [P, L], BF, name="vhs", tag="vraw", bufs=2)
                    vh = shared["vh"]
                    nc.vector.tensor_scalar_add(
                        vh[:, half * 1024:(half + 1) * 1024], pp[:], bias_sb[:])
                    if half == 1:
                        vh_cb(vh)
                return go

            if preloaded is None:
                units.append((4, load(0)))
                units.append((4, load(1)))
            for half in range(2):
                units.append((0, alloc_pp(half)))
                for kc in range(8):
                    units.append((2, mm_kc(half, kc)))
                units.append((0, evict_rope(half) if dst_sb is not None
                              else evict_v(half)))
            return units

        def vaug_transpose(b):
            # HW xbar transpose requires a contiguous destination; land in
            # vt then DVE-copy into the 65-stride vaug slots.
            def go(vh):
                vt = rots.tile([P, L], BF, name="vt", tag="vt", bufs=2)
                vt_r = vt.rearrange("p (h t u) -> p h t u", h=2, u=64)
                for h in range(2):
                    nc.sync.dma_start_transpose(
                        vt_r[:, h], vh[h * DK:(h + 1) * DK, :])
                for h in range(2):
                    va = vaug[b][h].rearrange(
                        "p (t u) -> p t u", u=65)[:, :, 0:64]
                    nc.vector.tensor_copy(va, vt_r[:, h])
            return go

        def attention(b, qb, budget=3):
            """512 q tokens; 16 kt tiles; 2 heads row-tiled on the PE."""
            q0 = qb * 512
            cp = cpp.tile([65, 1024], F32, name="cp", tag="cp")
            ex_prev = None
            for kt in range(NKT + 1):
                ex_cur = None
                if kt < NKT:
                    k0 = kt * P
                    sc = scp.tile([P, 1024], F32, name="sc", tag="sc")
                    nc.tensor.matmul(
                        sc[:, 0:512],
                        lhsT=kk_b[b][0:DK, k0:k0 + P],
                        rhs=qq_b[b][0:DK, q0:q0 + 512],
                        start=True, stop=True, skip_group_check=True)
                    nc.tensor.matmul(
                        sc[:, 512:1024],
                        lhsT=kk_b[b][DK:P, k0:k0 + P],
                        rhs=qq_b[b][DK:P, q0:q0 + 512],
                        start=True, stop=True, skip_group_check=True)
                    ex_cur = expp.tile([P, 1024], BF, name="ex", tag="ex")
                    nc.scalar.activation(ex_cur[:], sc[:], AF.Exp)
                    if debug_dumps and b == 0 and qb == 0 and kt == 0:
                        nc.sync.dma_start(dbg["dbg_exp"][:], ex_cur[:])
                if kt >= 1:
                    c = kt - 1
                    for h in range(2):
                        nc.tensor.matmul(
                            cp[:, h * 512:(h + 1) * 512],
                            lhsT=vaug[b][h][:, c * 65:(c + 1) * 65],
                            rhs=ex_prev[:, h * 512:(h + 1) * 512],
                            start=(c == 0), stop=(c == NKT - 1),
                            skip_group_check=True)
                ex_prev = ex_cur
                fill(budget)
            # normalize: copy cp out (frees the psum), move den row to
            # partition 0, reciprocal, broadcast, scale both heads.
            cps = smalls.tile([65, 1024], F32, name="cps", tag="cps")
            nc.vector.tensor_copy(cps[:], cp[:])
            den = smalls.tile([1, 1024], F32, name="den", tag="den", bufs=1)
            nc.sync.dma_start(den[:], cps[64:65, :])
            rec = smalls.tile([1, 1024], F32, name="rec", tag="rec", bufs=1)
            nc.vector.reciprocal_approx_fast(rec[:], den[:])
            if debug_dumps and b == 0 and qb == 0:
                nc.sync.dma_start(dbg["dbg_cps"][:], cps[:])
                nc.sync.dma_start(dbg["dbg_rec"][:], rec[:])
            bcs = smalls.tile([DK, 1024], F32, name="bcs", tag="bcs")
            nc.gpsimd.partition_broadcast(bcs[:], rec[:], channels=DK)
            nc.vector.tensor_mul(
                ctx_b[b][0:DK, q0:q0 + 512], cps[0:DK, 0:512], bcs[:, 0:512])
            h1s = h1p.tile([DK, 512], BF, name="h1s", tag="h1s")
            nc.vector.tensor_mul(h1s[:], cps[0:DK, 512:1024], bcs[:, 512:1024])
            nc.sync.dma_start(ctx_b[b][DK:P, q0:q0 + 512], h1s[:])

        def out_units(b, qb):
            units = []

            def po_unit(tb):
                def go():
                    t0 = qb * 512 + tb * P
                    po = pop.tile([P, D], F32, name="po", tag="pp")
                    for nb in range(2):
                        nc.tensor.matmul(
                            po[:, nb * 512:(nb + 1) * 512],
                            lhsT=ctx_b[b][:, t0:t0 + P],
                            rhs=wo_sb[:, nb * 512:(nb + 1) * 512],
                            start=True, stop=True, skip_group_check=True)
                    ob = outs.tile([P, D], BF, name="ob", tag="ob")
                    nc.vector.tensor_copy(ob[:], po[:])
                    nc.sync.dma_start(outp[b * L + t0:b * L + t0 + P, :], ob[:])
                return go

            for tb in range(4):
                units.append((2, po_unit(tb)))
            return units

        # ---------- program ----------
        # prefetch ALL batch-0 inputs up front (transfers stripe across the
        # DMA engines while the projections consume them in arrival order)
        k0_x = (load_half("k", 0, 0), load_half("k", 0, 1))
        wv_sb = load_w("wv_sb", wvT)
        bv_sb = load_b("bv_sb", bv_d)
        v0_x = (load_half("v", 0, 0), load_half("v", 0, 1))
        wq_sb = load_w("wq_sb", wqT)
        bq_sb = load_b("bq_sb", bq_d)
        q0_x = (load_half("q", 0, 0), load_half("q", 0, 1))
        wo_sb = const.tile([P, D], BF)
        nc.sync.dma_start(wo_sb[:], woT[:])

        for _, fn in proj_units("k", 0, wk_sb, bk_sb, dst_sb=kk_b[0],
                                preloaded=k0_x):
            fn()
        for _, fn in proj_units("v", 0, wv_sb, bv_sb, vh_cb=vaug_transpose(0),
                                preloaded=v0_x):
            fn()
        for _, fn in proj_units("q", 0, wq_sb, bq_sb, dst_sb=qq_b[0],
                                preloaded=q0_x):
            fn()

        # batch 0 attention; feed batch-1 projections + batch-0 out_proj
        # into the scalar-bound loop as fillers.
        for qb in range(NQB):
            attention(0, qb)
            fillers.extend(out_units(0, qb))
            if qb == 0:
                fillers.extend(proj_units("k", 1, wk_sb, bk_sb, dst_sb=kk_b[1]))
                fillers.extend(proj_units("v", 1, wv_sb, bv_sb,
                                          vh_cb=vaug_transpose(1)))
            elif qb == 1:
                fillers.extend(proj_units("q", 1, wq_sb, bq_sb, dst_sb=qq_b[1]))
        flush()  # b1 attention depends on b1 projections: drain first

        for qb in range(NQB):
            attention(1, qb)
            fillers.extend(out_units(1, qb))
        flush()

        if debug_dumps:
            nc.sync.dma_start(dbg["dbg_qq"][:], qq_b[0][:])
            nc.sync.dma_start(dbg["dbg_kk"][:], kk_b[0][:])
            nc.sync.dma_start(dbg["dbg_vaug0"][:], vaug[0][0][:])
            nc.sync.dma_start(dbg["dbg_vaug1"][:], vaug[0][1][:])
            nc.sync.dma_start(dbg["dbg_ctx"][:], ctx_b[0][:])

    return nc


def _rope_tables():
    """Host-built RoPE tables [d, t], 2 heads stacked, sign-folded sin."""
    inv_freq = 1.0 / (ROPE_BASE ** (np.arange(0, DK, 2, dtype=np.float64) / DK))
    t = np.arange(L, dtype=np.float64)
    ang = np.outer(t, inv_freq)               # [L, 32]
    emb = np.concatenate([ang, ang], axis=1)  # [L, 64]
    cos = np.cos(emb).T.astype(np.float32)    # [64, L]
    sin = np.sin(emb).T.astype(np.float32)
    sin_folded = sin.copy()
    sin_folded[:32] *= -1.0
    bf = ml_dtypes.bfloat16
    cos2 = np.concatenate([cos, cos], axis=0)                # [128, L]
    sin2 = np.concatenate([sin_folded, sin_folded], axis=0)  # [128, L]
    return cos2.astype(bf), sin2.astype(bf)


def host_in_maps(q, k, v, Wq, bq, Wk, bk, Wv, bv, Wo):
    """Per-core input maps (the 1/sqrt(dk) scale is folded into Wq/bq)."""
    bf = ml_dtypes.bfloat16
    qT = np.ascontiguousarray(np.asarray(q).reshape(TOK, D).T).astype(bf)
    kT = np.ascontiguousarray(np.asarray(k).reshape(TOK, D).T).astype(bf)
    vT = np.ascontiguousarray(np.asarray(v).reshape(TOK, D).T).astype(bf)
    cos_t, sin_t = _rope_tables()
    scale = 1.0 / np.sqrt(DK)
    in_maps = []
    for c in range(NCORES):
        hs = slice(c * PD, (c + 1) * PD)
        in_maps.append({
            "qT": qT, "kT": kT, "vT": vT,
            "wqT": np.ascontiguousarray(
                (np.asarray(Wq)[hs, :] * scale).T).astype(bf),
            "wkT": np.ascontiguousarray(np.asarray(Wk)[hs, :].T).astype(bf),
            "wvT": np.ascontiguousarray(np.asarray(Wv)[hs, :].T).astype(bf),
            "woT": np.ascontiguousarray(np.asarray(Wo)[:, hs].T).astype(bf),
            "bq": (np.asarray(bq)[hs] * scale).astype(np.float32).reshape(PD, 1),
            "bk": np.asarray(bk[hs], np.float32).reshape(PD, 1),
            "bv": np.asarray(bv[hs], np.float32).reshape(PD, 1),
            "cos_t": cos_t, "sin_t": sin_t,
        })
    return in_maps


_NC_CACHE = {}


def _get_nc():
    if "nc" not in _NC_CACHE:
        nc = build_nc()
        nc.finalize()
        _NC_CACHE["nc"] = nc
    return _NC_CACHE["nc"]


def kernel(q, k, v, Wq, bq, Wk, bk, Wv, bv, Wo, bo):
    assert q.shape == (B, L, D) and k.shape == (B, L, D) and v.shape == (B, L, D)
    in_maps = host_in_maps(q, k, v, Wq, bq, Wk, bk, Wv, bv, Wo)
    nc = _get_nc()
    res = run_bass_kernel_spmd(nc, in_maps, list(range(NCORES)))
    out = np.zeros((TOK, D), np.float64)
    for r in res.results:
        out += r["outp"].astype(np.float64)
    out += np.asarray(bo, np.float64)[None, :]
    return out.astype(np.float32).reshape(B, L, D)


# revision 21
# speedup vs baseline: 1.4755x; 1.0335x over previous
"""Trainium2 Bass kernel for MultiHeadAttention with RoPE.

Problem: B=2, L=2048, d_model=1024, 16 heads, d_k=64, fp32 in/out.

Sharding (8 cores): tensor-parallel over heads — core c owns heads
{2c, 2c+1}, i.e. a 128-wide slice of the projection output dims.  Every
core reads the full q/k/v activations (transposed + bf16 on host), its
own 128-row slice of Wq/Wk/Wv (pre-transposed; Wq/bq pre-scaled by
1/sqrt(dk)) and the matching 128 columns of Wo.  Each core computes its
heads' attention output and a partial d_model output projection; the
host sums the 8 partials and adds bo.

Per-core pipeline (bf16 matmuls, fp32 PSUM):
  1. QKV projections [128 pd, 1024 tok] halves; bias-add + bf16 evict on
     DVE; RoPE via partition-swap DMAs + 3 DVE ops (sign folded in sin
     table, 1/sqrt(dk) folded into Wq).
  2. V-heads transposed to [kt, dim] layout by ONE dma_start_transpose
     per (batch, head) into a 65-wide-stride "vaug" buffer whose 65th
     column is ones.
  3. Scores: per 128-kt tile, the two heads run CONCURRENTLY on the PE
     as K=64 row-tiles (tile_position (0,0) / (64,0) auto-derived).
  4. exp on ScalarE ([128, 1024] per kt tile covering both heads).
  5. ctx: lhsT = vaug [128 kt, 65] per head; row 64 accumulates the
     softmax denominator for free (M=65 stationary).
  6. normalize: DVE copy of cp, denominator row -> partition 0 via DMA,
     reciprocal + GpSimd broadcast + DVE muls; h1 ctx shifted to
     partitions 64:127 by a small DMA.
  7. out_proj [tok, 1024] = ctx (stationary) @ WoT slice.
Emission interleaves next-batch projections and out_proj matmuls into
the (ScalarE-bound) attention loops as "fillers" so the PE never idles
long and HAM stays at full clock.  All activation buffers are
per-batch tiles so filler writes never create false WAR dependencies
against the running attention.
"""

import collections
import numpy as np
import ml_dtypes

import concourse.bass as bass
import concourse.mybir as mybir
import concourse.tile as tile
from concourse import bacc
from concourse.bass_utils import run_bass_kernel_spmd

BF = mybir.dt.bfloat16
F32 = mybir.dt.float32
AF = mybir.ActivationFunctionType

NCORES = 8
B = 2
L = 2048
D = 1024          # d_model
H = 16            # heads
DK = 64           # head dim
HPC = H // NCORES  # heads per core = 2
PD = HPC * DK      # projection dims per core = 128
TOK = B * L        # 4096 tokens
P = 128
NKT = L // P       # 16 kt tiles per batch
NQB = 4            # 512-token q blocks per batch

ROPE_BASE = 10000.0


def build_nc(debug_dumps=False):
    """Build the single-core Bass program (SPMD: same program, per-core data)."""
    from contextlib import ExitStack

    nc = bacc.Bacc("TRN2", target_bir_lowering=False, debug=False)
    dbg = {}
    if debug_dumps:
        for nm, shp, dt in [
            ("dbg_qq", [P, L], BF), ("dbg_kk", [P, L], BF),
            ("dbg_vaug0", [P, NKT * 65], BF), ("dbg_vaug1", [P, NKT * 65], BF),
            ("dbg_exp", [P, 1024], BF), ("dbg_cps", [65, 1024], F32),
            ("dbg_rec", [1, 1024], F32), ("dbg_ctx", [P, L], BF),
        ]:
            dbg[nm] = nc.dram_tensor(nm, shp, dt, kind="ExternalOutput").ap()

    # ---- DRAM I/O ----
    # Host pre-arranges everything partition-contiguous so each DMA is 128
    # descriptors of big contiguous chunks (dispatch cost ~ descriptor count).
    # xH[p, (b, half, a, t)]: token t of half `half` of batch b, dim a*128+p.
    qH = nc.dram_tensor("qH", [P, B * 2 * 8 * 1024], BF, kind="ExternalInput").ap()
    kH = nc.dram_tensor("kH", [P, B * 2 * 8 * 1024], BF, kind="ExternalInput").ap()
    vH = nc.dram_tensor("vH", [P, B * 2 * 8 * 1024], BF, kind="ExternalInput").ap()
    # wH[p, (a, m)]: weight row a*128+p, output dim m.
    wqH = nc.dram_tensor("wqH", [P, 8 * P], BF, kind="ExternalInput").ap()
    wkH = nc.dram_tensor("wkH", [P, 8 * P], BF, kind="ExternalInput").ap()
    wvH = nc.dram_tensor("wvH", [P, 8 * P], BF, kind="ExternalInput").ap()
    woT = nc.dram_tensor("woT", [PD, D], BF, kind="ExternalInput").ap()
    bias_d = nc.dram_tensor("biases", [PD, 3], F32, kind="ExternalInput").ap()
    cos_d = nc.dram_tensor("cos_t", [P, L], BF, kind="ExternalInput").ap()
    sin_d = nc.dram_tensor("sin_t", [P, L], BF, kind="ExternalInput").ap()
    outp = nc.dram_tensor("outp", [TOK, D], BF, kind="ExternalOutput").ap()

    xT = {"q": qH, "k": kH, "v": vH}

    with tile.TileContext(nc) as tc, ExitStack() as ctx:
        const = ctx.enter_context(tc.tile_pool(name="const", bufs=1))
        persist = ctx.enter_context(tc.tile_pool(name="persist", bufs=1))
        stage = ctx.enter_context(tc.tile_pool(name="stage", bufs=5))
        raws = ctx.enter_context(tc.tile_pool(name="raws", bufs=2))
        rots = ctx.enter_context(tc.tile_pool(name="rots", bufs=2))
        expp = ctx.enter_context(tc.tile_pool(name="expp", bufs=3))
        outs = ctx.enter_context(tc.tile_pool(name="outs", bufs=3))
        smalls = ctx.enter_context(tc.tile_pool(name="smalls", bufs=2))
        h1p = ctx.enter_context(tc.tile_pool(name="h1p", bufs=2))
        # PSUM: scores 2 tiles x 2 banks + ctx 2 banks + proj/out 2 banks = 8
        scp = ctx.enter_context(tc.tile_pool(name="scp", bufs=2, space="PSUM"))
        cpp = ctx.enter_context(tc.tile_pool(name="cpp", bufs=1, space="PSUM"))
        pop = ctx.enter_context(tc.tile_pool(name="pop", bufs=1, space="PSUM"))

        # ---- constants (emitted in first-use order) ----
        def load_w(name, w_d):
            w_sb = const.tile([P, 8 * P], BF, name=name)
            nc.sync.dma_start(w_sb[:], w_d[:])
            return w_sb

        wk_sb = load_w("wk_sb", wkH)
        bias_sb3 = const.tile([P, 3], F32, name="bias_sb3")
        nc.sync.dma_start(bias_sb3[:], bias_d[:])
        bq_sb = bias_sb3[:, 0:1]
        bk_sb = bias_sb3[:, 1:2]
        bv_sb = bias_sb3[:, 2:3]
        cos_sb = const.tile([P, L], BF)
        nc.sync.dma_start(cos_sb[:], cos_d[:])
        sin_sb = const.tile([P, L], BF)
        nc.sync.dma_start(sin_sb[:], sin_d[:])

        # per-batch persistent activations [128 dims, 2048 tok]
        qq_b = [persist.tile([P, L], BF, name=f"qq{b}") for b in range(B)]
        kk_b = [persist.tile([P, L], BF, name=f"kk{b}") for b in range(B)]
        ctx_b = [persist.tile([P, L], BF, name=f"ctx{b}") for b in range(B)]
        # vaug[b][h]: 16 slots of [128 kt, 65]; cols 0:64 = v dims
        # (t-major: slot t partition p holds token t*128+p), col 64 = ones.
        vaug = [[persist.tile([P, NKT * 65], BF, name=f"vaug_{b}_{h}")
                 for h in range(2)] for b in range(B)]
        for b in range(B):
            for h in range(2):
                va = vaug[b][h].rearrange("p (t u) -> p t u", u=65)
                nc.vector.memset(va[:, :, 64:65], 1.0)

        # ---------- filler machinery ----------
        fillers = collections.deque()

        def fill(budget):
            while fillers and budget > 0:
                cost, fn = fillers.popleft()
                fn()
                budget -= cost

        def flush():
            while fillers:
                fillers.popleft()[1]()

        # ---------- phase helpers ----------
        def load_half(which, b, half):
            """One 2MB DMA, per-partition contiguous: a 1024-token half."""
            xt = stage.tile([P, 8 * 1024], BF, name="xstage", tag="stage")
            j = (b * 2 + half) * 8192
            nc.sync.dma_start(xt[:], xT[which][:, j:j + 8192])
            return xt

        def proj_units(which, b, w_sb, bias_sb, dst_sb=None, vh_cb=None,
                       preloaded=None):
            """Filler units projecting batch b (2 halves of 1024 tokens).

            dst_sb given -> rope into it (q/k).  vh_cb given -> v path:
            evict to a fresh vh tile, call vh_cb(vh_tile) when done.
            preloaded: stage tiles already loaded (batch-0 prefetch).
            """
            units = []
            shared = {}
            if preloaded is not None:
                shared[("x", 0)], shared[("x", 1)] = preloaded

            def load(half):
                def go():
                    shared[("x", half)] = load_half(which, b, half)
                return go

            def alloc_pp(half):
                def go():
                    shared[("pp", half)] = pop.tile(
                        [P, 1024], F32, name="pp", tag="pp")
                return go

            def mm_kc(half, kc):
                def go():
                    xt = shared[("x", half)].rearrange("p (a t) -> p a t", a=8)
                    pp = shared[("pp", half)]
                    for nb in range(2):
                        nc.tensor.matmul(
                            pp[:, nb * 512:(nb + 1) * 512],
                            lhsT=w_sb[:, kc * P:(kc + 1) * P],
                            rhs=xt[:, kc, nb * 512:(nb + 1) * 512],
                            start=(kc == 0), stop=(kc == 7),
                        )
                return go

            def evict_rope(half):
                def go():
                    pp = shared[("pp", half)]
                    raw = raws.tile([P, 1024], BF, name="raw", tag="raw")
                    nc.vector.tensor_scalar_add(raw[:], pp[:], bias_sb[:])
                    rot = rots.tile([P, 1024], BF, name="rot", tag="rot")
                    for h in range(2):
                        r0 = h * DK
                        nc.sync.dma_start(rot[r0:r0 + 32, :],
                                          raw[r0 + 32:r0 + 64, :])
                        nc.sync.dma_start(rot[r0 + 32:r0 + 64, :],
                                          raw[r0:r0 + 32, :])
                    cs = slice(half * 1024, (half + 1) * 1024)
                    nc.vector.tensor_mul(raw[:], raw[:], cos_sb[:, cs])
                    nc.vector.tensor_mul(rot[:], rot[:], sin_sb[:, cs])
                    nc.vector.tensor_add(dst_sb[:, cs], raw[:], rot[:])
                return go

            def evict_v(half):
                def go():
                    pp = shared[("pp", half)]
                    if "vh" not in shared:
                        shared["vh"] = raws.tile(
                            [P, L], BF, name="vhs", tag="vraw", bufs=2)
                    vh = shared["vh"]
                    nc.vector.tensor_scalar_add(
                        vh[:, half * 1024:(half + 1) * 1024], pp[:], bias_sb[:])
                    if half == 1:
                        vh_cb(vh)
                return go

            if preloaded is None:
                units.append((4, load(0)))
                units.append((4, load(1)))
            for half in range(2):
                units.append((0, alloc_pp(half)))
                for kc in range(8):
                    units.append((2, mm_kc(half, kc)))
                units.append((0, evict_rope(half) if dst_sb is not None
                              else evict_v(half)))
            return units

        def vaug_transpose(b):
            # HW xbar transpose requires a contiguous destination; land in
            # vt then DVE-copy into the 65-stride vaug slots.
            def go(vh):
                vt = rots.tile([P, L], BF, name="vt", tag="vt", bufs=2)
                vt_r = vt.rearrange("p (h t u) -> p h t u", h=2, u=64)
                for h in range(2):
                    nc.sync.dma_start_transpose(
                        vt_r[:, h], vh[h * DK:(h + 1) * DK, :])
                for h in range(2):
                    va = vaug[b][h].rearrange(
                        "p (t u) -> p t u", u=65)[:, :, 0:64]
                    nc.vector.tensor_copy(va, vt_r[:, h])
            return go

        def attention(b, qb, budget=3):
            """512 q tokens; 16 kt tiles; 2 heads row-tiled on the PE."""
            q0 = qb * 512
            cp = cpp.tile([65, 1024], F32, name="cp", tag="cp")
            ex_prev = None
            for kt in range(NKT + 1):
                ex_cur = None
                if kt < NKT:
                    k0 = kt * P
                    sc = scp.tile([P, 1024], F32, name="sc", tag="sc")
                    nc.tensor.matmul(
                        sc[:, 0:512],
                        lhsT=kk_b[b][0:DK, k0:k0 + P],
                        rhs=qq_b[b][0:DK, q0:q0 + 512],
                        start=True, stop=True, skip_group_check=True)
                    nc.tensor.matmul(
                        sc[:, 512:1024],
                        lhsT=kk_b[b][DK:P, k0:k0 + P],
                        rhs=qq_b[b][DK:P, q0:q0 + 512],
                        start=True, stop=True, skip_group_check=True)
                    ex_cur = expp.tile([P, 1024], BF, name="ex", tag="ex")
                    nc.scalar.activation(ex_cur[:], sc[:], AF.Exp)
                    if debug_dumps and b == 0 and qb == 0 and kt == 0:
                        nc.sync.dma_start(dbg["dbg_exp"][:], ex_cur[:])
                if kt >= 1:
                    c = kt - 1
                    for h in range(2):
                        nc.tensor.matmul(
                            cp[:, h * 512:(h + 1) * 512],
                            lhsT=vaug[b][h][:, c * 65:(c + 1) * 65],
                            rhs=ex_prev[:, h * 512:(h + 1) * 512],
                            start=(c == 0), stop=(c == NKT - 1),
                            skip_group_check=True)
                ex_prev = ex_cur
                fill(budget)
            # normalize: copy cp out (frees the psum), move den row to
            # partition 0, reciprocal, broadcast, scale both heads.
            cps = smalls.tile([65, 1024], F32, name="cps", tag="cps")
            nc.vector.tensor_copy(cps[:], cp[:])
            den = smalls.tile([1, 1024], F32, name="den", tag="den", bufs=1)
            nc.sync.dma_start(den[:], cps[64:65, :])
            rec = smalls.tile([1, 1024], F32, name="rec", tag="rec", bufs=1)
            nc.vector.reciprocal_approx_fast(rec[:], den[:])
            if debug_dumps and b == 0 and qb == 0:
                nc.sync.dma_start(dbg["dbg_cps"][:], cps[:])
                nc.sync.dma_start(dbg["dbg_rec"][:], rec[:])
            bcs = smalls.tile([DK, 1024], F32, name="bcs", tag="bcs")
            nc.gpsimd.partition_broadcast(bcs[:], rec[:], channels=DK)
            nc.vector.tensor_mul(
                ctx_b[b][0:DK, q0:q0 + 512], cps[0:DK, 0:512], bcs[:, 0:512])
            h1s = h1p.tile([DK, 512], BF, name="h1s", tag="h1s")
            nc.vector.tensor_mul(h1s[:], cps[0:DK, 512:1024], bcs[:, 512:1024])
            nc.sync.dma_start(ctx_b[b][DK:P, q0:q0 + 512], h1s[:])

        def out_units(b, qb):
            units = []

            def po_unit(tb):
                def go():
                    t0 = qb * 512 + tb * P
                    po = pop.tile([P, D], F32, name="po", tag="pp")
                    for nb in range(2):
                        nc.tensor.matmul(
                            po[:, nb * 512:(nb + 1) * 512],
                            lhsT=ctx_b[b][:, t0:t0 + P],
                            rhs=wo_sb[:, nb * 512:(nb + 1) * 512],
                            start=True, stop=True, skip_group_check=True)
                    ob = outs.tile([P, D], BF, name="ob", tag="ob")
                    nc.vector.tensor_copy(ob[:], po[:])
                    nc.sync.dma_start(outp[b * L + t0:b * L + t0 + P, :], ob[:])
                return go

            for tb in range(4):
                units.append((2, po_unit(tb)))
            return units

        # ---------- program ----------
        # prefetch ALL batch-0 inputs up front (transfers stripe across the
        # DMA engines while the projections consume them in arrival order)
        k0_x = (load_half("k", 0, 0), load_half("k", 0, 1))
        wv_sb = load_w("wv_sb", wvH)
        v0_x = (load_half("v", 0, 0), load_half("v", 0, 1))
        wq_sb = load_w("wq_sb", wqH)
        q0_x = (load_half("q", 0, 0), load_half("q", 0, 1))
        wo_sb = const.tile([P, D], BF)
        nc.sync.dma_start(wo_sb[:], woT[:])

        for _, fn in proj_units("k", 0, wk_sb, bk_sb, dst_sb=kk_b[0],
                                preloaded=k0_x):
            fn()
        for _, fn in proj_units("v", 0, wv_sb, bv_sb, vh_cb=vaug_transpose(0),
                                preloaded=v0_x):
            fn()
        for _, fn in proj_units("q", 0, wq_sb, bq_sb, dst_sb=qq_b[0],
                                preloaded=q0_x):
            fn()

        # batch 0 attention; feed batch-1 projections + batch-0 out_proj
        # into the scalar-bound loop as fillers.
        for qb in range(NQB):
            attention(0, qb)
            fillers.extend(out_units(0, qb))
            if qb == 0:
                fillers.extend(proj_units("k", 1, wk_sb, bk_sb, dst_sb=kk_b[1]))
                fillers.extend(proj_units("v", 1, wv_sb, bv_sb,
                                          vh_cb=vaug_transpose(1)))
            elif qb == 1:
                fillers.extend(proj_units("q", 1, wq_sb, bq_sb, dst_sb=qq_b[1]))
        flush()  # b1 attention depends on b1 projections: drain first

        for qb in range(NQB):
            attention(1, qb)
            fillers.extend(out_units(1, qb))
        flush()

        if debug_dumps:
            nc.sync.dma_start(dbg["dbg_qq"][:], qq_b[0][:])
            nc.sync.dma_start(dbg["dbg_kk"][:], kk_b[0][:])
            nc.sync.dma_start(dbg["dbg_vaug0"][:], vaug[0][0][:])
            nc.sync.dma_start(dbg["dbg_vaug1"][:], vaug[0][1][:])
            nc.sync.dma_start(dbg["dbg_ctx"][:], ctx_b[0][:])

    return nc


def _rope_tables():
    """Host-built RoPE tables [d, t], 2 heads stacked, sign-folded sin."""
    inv_freq = 1.0 / (ROPE_BASE ** (np.arange(0, DK, 2, dtype=np.float64) / DK))
    t = np.arange(L, dtype=np.float64)
    ang = np.outer(t, inv_freq)               # [L, 32]
    emb = np.concatenate([ang, ang], axis=1)  # [L, 64]
    cos = np.cos(emb).T.astype(np.float32)    # [64, L]
    sin = np.sin(emb).T.astype(np.float32)
    sin_folded = sin.copy()
    sin_folded[:32] *= -1.0
    bf = ml_dtypes.bfloat16
    cos2 = np.concatenate([cos, cos], axis=0)                # [128, L]
    sin2 = np.concatenate([sin_folded, sin_folded], axis=0)  # [128, L]
    return cos2.astype(bf), sin2.astype(bf)


def _xh(x):
    """[B, L, D] -> [128, B*2*8*1024]: xH[p, (b, hf, a, t)] = x[b, hf*1024+t,
    a*128+p] — every (b, half) load is per-partition contiguous."""
    bf = ml_dtypes.bfloat16
    xr = np.asarray(x).reshape(B, 2, 1024, 8, P).transpose(4, 0, 1, 3, 2)
    return np.ascontiguousarray(xr.reshape(P, B * 2 * 8 * 1024)).astype(bf)


def _wh(w_slice):
    """[PD, D] weight slice -> [128, 8*128]: wH[p, (a, m)] = W.T[a*128+p, m]."""
    bf = ml_dtypes.bfloat16
    wr = np.ascontiguousarray(w_slice.T).reshape(8, P, PD).transpose(1, 0, 2)
    return np.ascontiguousarray(wr.reshape(P, 8 * PD)).astype(bf)


def host_in_maps(q, k, v, Wq, bq, Wk, bk, Wv, bv, Wo):
    """Per-core input maps (the 1/sqrt(dk) scale is folded into Wq/bq)."""
    bf = ml_dtypes.bfloat16
    qh, kh, vh = _xh(q), _xh(k), _xh(v)
    cos_t, sin_t = _rope_tables()
    scale = 1.0 / np.sqrt(DK)
    in_maps = []
    for c in range(NCORES):
        hs = slice(c * PD, (c + 1) * PD)
        biases = np.stack([
            np.asarray(bq)[hs] * scale, np.asarray(bk)[hs],
            np.asarray(bv)[hs]], axis=1).astype(np.float32)
        in_maps.append({
            "qH": qh, "kH": kh, "vH": vh,
            "wqH": _wh(np.asarray(Wq)[hs, :] * scale),
            "wkH": _wh(np.asarray(Wk)[hs, :]),
            "wvH": _wh(np.asarray(Wv)[hs, :]),
            "woT": np.ascontiguousarray(np.asarray(Wo)[:, hs].T).astype(bf),
            "biases": biases,
            "cos_t": cos_t, "sin_t": sin_t,
        })
    return in_maps


_NC_CACHE = {}


def _get_nc():
    if "nc" not in _NC_CACHE:
        nc = build_nc()
        nc.finalize()
        _NC_CACHE["nc"] = nc
    return _NC_CACHE["nc"]


def kernel(q, k, v, Wq, bq, Wk, bk, Wv, bv, Wo, bo):
    assert q.shape == (B, L, D) and k.shape == (B, L, D) and v.shape == (B, L, D)
    in_maps = host_in_maps(q, k, v, Wq, bq, Wk, bk, Wv, bv, Wo)
    nc = _get_nc()
    res = run_bass_kernel_spmd(nc, in_maps, list(range(NCORES)))
    out = np.zeros((TOK, D), np.float64)
    for r in res.results:
        out += r["outp"].astype(np.float64)
    out += np.asarray(bo, np.float64)[None, :]
    return out.astype(np.float32).reshape(B, L, D)


# revision 31
# speedup vs baseline: 1.4756x; 1.0001x over previous
"""Trainium2 Bass kernel for MultiHeadAttention with RoPE.

Problem: B=2, L=2048, d_model=1024, 16 heads, d_k=64, fp32 in/out.

Sharding (8 cores): tensor-parallel over heads — core c owns heads
{2c, 2c+1}, i.e. a 128-wide slice of the projection output dims.  Every
core reads the full q/k/v activations (transposed + bf16 on host), its
own 128-row slice of Wq/Wk/Wv (pre-transposed; Wq/bq pre-scaled by
1/sqrt(dk)) and the matching 128 columns of Wo.  Each core computes its
heads' attention output and a partial d_model output projection; the
host sums the 8 partials and adds bo.

Per-core pipeline (bf16 matmuls, fp32 PSUM):
  1. QKV projections [128 pd, 1024 tok] halves; bias-add + bf16 evict on
     DVE; RoPE via partition-swap DMAs + 3 DVE ops (sign folded in sin
     table, 1/sqrt(dk) folded into Wq).
  2. V-heads transposed to [kt, dim] layout by ONE dma_start_transpose
     per (batch, head) into a 65-wide-stride "vaug" buffer whose 65th
     column is ones.
  3. Scores: per 128-kt tile, the two heads run CONCURRENTLY on the PE
     as K=64 row-tiles (tile_position (0,0) / (64,0) auto-derived).
  4. exp on ScalarE ([128, 1024] per kt tile covering both heads).
  5. ctx: lhsT = vaug [128 kt, 65] per head; row 64 accumulates the
     softmax denominator for free (M=65 stationary).
  6. normalize: DVE copy of cp, denominator row -> partition 0 via DMA,
     reciprocal + GpSimd broadcast + DVE muls; h1 ctx shifted to
     partitions 64:127 by a small DMA.
  7. out_proj [tok, 1024] = ctx (stationary) @ WoT slice.
Emission interleaves next-batch projections and out_proj matmuls into
the (ScalarE-bound) attention loops as "fillers" so the PE never idles
long and HAM stays at full clock.  All activation buffers are
per-batch tiles so filler writes never create false WAR dependencies
against the running attention.
"""

import collections
import numpy as np
import ml_dtypes

import concourse.bass as bass
import concourse.mybir as mybir
import concourse.tile as tile
from concourse import bacc
from concourse.bass_utils import run_bass_kernel_spmd

BF = mybir.dt.bfloat16
F32 = mybir.dt.float32
AF = mybir.ActivationFunctionType

NCORES = 8
B = 2
L = 2048
D = 1024          # d_model
H = 16            # heads
DK = 64           # head dim
HPC = H // NCORES  # heads per core = 2
PD = HPC * DK      # projection dims per core = 128
TOK = B * L        # 4096 tokens
P = 128
NKT = L // P       # 16 kt tiles per batch
NQB = 4            # 512-token q blocks per batch

ROPE_BASE = 10000.0


def build_nc(debug_dumps=False):
    """Build the single-core Bass program (SPMD: same program, per-core data)."""
    from contextlib import ExitStack

    nc = bacc.Bacc("TRN2", target_bir_lowering=False, debug=False)
    dbg = {}
    if debug_dumps:
        for nm, shp, dt in [
            ("dbg_qq", [P, L], BF), ("dbg_kk", [P, L], BF),
            ("dbg_vaug0", [P, NKT * 65], BF), ("dbg_vaug1", [P, NKT * 65], BF),
            ("dbg_exp", [P, 1024], BF), ("dbg_cps", [65, 1024], F32),
            ("dbg_rec", [1, 1024], F32), ("dbg_ctx", [P, L], BF),
        ]:
            dbg[nm] = nc.dram_tensor(nm, shp, dt, kind="ExternalOutput").ap()

    # ---- DRAM I/O ----
    # Host pre-arranges everything partition-contiguous so each DMA is 128
    # descriptors of big contiguous chunks (dispatch cost ~ descriptor count).
    # xH[p, (b, half, a, t)]: token t of half `half` of batch b, dim a*128+p.
    qH = nc.dram_tensor("qH", [P, B * 2 * 8 * 1024], BF, kind="ExternalInput").ap()
    kH = nc.dram_tensor("kH", [P, B * 2 * 8 * 1024], BF, kind="ExternalInput").ap()
    vH = nc.dram_tensor("vH", [P, B * 2 * 8 * 1024], BF, kind="ExternalInput").ap()
    # wH[p, (a, m)]: weight row a*128+p, output dim m.
    wqH = nc.dram_tensor("wqH", [P, 8 * P], BF, kind="ExternalInput").ap()
    wkH = nc.dram_tensor("wkH", [P, 8 * P], BF, kind="ExternalInput").ap()
    wvH = nc.dram_tensor("wvH", [P, 8 * P], BF, kind="ExternalInput").ap()
    woT = nc.dram_tensor("woT", [PD, D], BF, kind="ExternalInput").ap()
    bias_d = nc.dram_tensor("biases", [PD, 3], F32, kind="ExternalInput").ap()
    cos_d = nc.dram_tensor("cos_t", [P, L], BF, kind="ExternalInput").ap()
    sin_d = nc.dram_tensor("sin_t", [P, L], BF, kind="ExternalInput").ap()
    outp = nc.dram_tensor("outp", [TOK, D], BF, kind="ExternalOutput").ap()

    xT = {"q": qH, "k": kH, "v": vH}

    with tile.TileContext(nc) as tc, ExitStack() as ctx:
        const = ctx.enter_context(tc.tile_pool(name="const", bufs=1))
        persist = ctx.enter_context(tc.tile_pool(name="persist", bufs=1))
        stage = ctx.enter_context(tc.tile_pool(name="stage", bufs=5))
        raws = ctx.enter_context(tc.tile_pool(name="raws", bufs=2))
        rots = ctx.enter_context(tc.tile_pool(name="rots", bufs=2))
        expp = ctx.enter_context(tc.tile_pool(name="expp", bufs=3))
        outs = ctx.enter_context(tc.tile_pool(name="outs", bufs=3))
        smalls = ctx.enter_context(tc.tile_pool(name="smalls", bufs=2))
        h1p = ctx.enter_context(tc.tile_pool(name="h1p", bufs=2))
        # PSUM: scores 2 tiles x 2 banks + ctx 2 banks + proj/out 2 banks = 8
        scp = ctx.enter_context(tc.tile_pool(name="scp", bufs=2, space="PSUM"))
        cpp = ctx.enter_context(tc.tile_pool(name="cpp", bufs=1, space="PSUM"))
        pop = ctx.enter_context(tc.tile_pool(name="pop", bufs=1, space="PSUM"))

        # ---- constants (emitted in first-use order) ----
        def load_w(name, w_d):
            w_sb = const.tile([P, 8 * P], BF, name=name)
            nc.sync.dma_start(w_sb[:], w_d[:])
            return w_sb

        wk_sb = load_w("wk_sb", wkH)
        bias_sb3 = const.tile([P, 3], F32, name="bias_sb3")
        nc.sync.dma_start(bias_sb3[:], bias_d[:])
        bq_sb = bias_sb3[:, 0:1]
        bk_sb = bias_sb3[:, 1:2]
        bv_sb = bias_sb3[:, 2:3]
        cos_sb = const.tile([P, L], BF)
        nc.sync.dma_start(cos_sb[:], cos_d[:])
        sin_sb = const.tile([P, L], BF)
        nc.sync.dma_start(sin_sb[:], sin_d[:])

        # per-(batch, token-half) persistent activations [128 dims, 1024 tok]
        # (split so scores never wait on the other half's rope chain)
        qq_h = [[persist.tile([P, 1024], BF, name=f"qq{b}_{hf}")
                 for hf in range(2)] for b in range(B)]
        kk_h = [[persist.tile([P, 1024], BF, name=f"kk{b}_{hf}")
                 for hf in range(2)] for b in range(B)]
        ctx_b = [persist.tile([P, L], BF, name=f"ctx{b}") for b in range(B)]
        # vaug[b][h]: 16 slots of [128 kt, 65]; cols 0:64 = v dims
        # (t-major: slot t partition p holds token t*128+p), col 64 = ones.
        vaug = [[persist.tile([P, NKT * 65], BF, name=f"vaug_{b}_{h}")
                 for h in range(2)] for b in range(B)]
        for b in range(B):
            for h in range(2):
                va = vaug[b][h].rearrange("p (t u) -> p t u", u=65)
                nc.vector.memset(va[:, :, 64:65], 1.0)

        # ---------- filler machinery ----------
        fillers = collections.deque()

        def fill(budget):
            while fillers and budget > 0:
                cost, fn = fillers.popleft()
                fn()
                budget -= cost

        def flush():
            while fillers:
                fillers.popleft()[1]()

        # ---------- phase helpers ----------
        def load_half(which, b, half):
            """One 2MB DMA, per-partition contiguous: a 1024-token half."""
            xt = stage.tile([P, 8 * 1024], BF, name="xstage", tag="stage")
            j = (b * 2 + half) * 8192
            nc.sync.dma_start(xt[:], xT[which][:, j:j + 8192])
            return xt

        def proj_units(which, b, w_sb, bias_sb, dst_sb=None, vh_cb=None,
                       preloaded=None):
            """Filler units projecting batch b (2 halves of 1024 tokens).

            dst_sb given -> rope into it (q/k).  vh_cb given -> v path:
            evict to a fresh vh tile, call vh_cb(vh_tile) when done.
            preloaded: stage tiles already loaded (batch-0 prefetch).
            """
            units = []
            shared = {}
            if preloaded is not None:
                shared[("x", 0)], shared[("x", 1)] = preloaded

            def load(half):
                def go():
                    shared[("x", half)] = load_half(which, b, half)
                return go

            def alloc_pp(half):
                def go():
                    shared[("pp", half)] = pop.tile(
                        [P, 1024], F32, name="pp", tag="pp")
                return go

            def mm_kc(half, kc):
                def go():
                    xt = shared[("x", half)].rearrange("p (a t) -> p a t", a=8)
                    pp = shared[("pp", half)]
                    for nb in range(2):
                        nc.tensor.matmul(
                            pp[:, nb * 512:(nb + 1) * 512],
                            lhsT=w_sb[:, kc * P:(kc + 1) * P],
                            rhs=xt[:, kc, nb * 512:(nb + 1) * 512],
                            start=(kc == 0), stop=(kc == 7),
                        )
                return go

            def evict_rot(half):
                def go():
                    pp = shared[("pp", half)]
                    raw = raws.tile([P, 1024], BF, name="raw", tag="raw")
                    nc.vector.tensor_scalar_add(raw[:], pp[:], bias_sb[:])
                    rot = rots.tile([P, 1024], BF, name="rot", tag="rot")
                    for h in range(2):
                        r0 = h * DK
                        nc.sync.dma_start(rot[r0:r0 + 32, :],
                                          raw[r0 + 32:r0 + 64, :])
                        nc.sync.dma_start(rot[r0 + 32:r0 + 64, :],
                                          raw[r0:r0 + 32, :])
                    shared[("rr", half)] = (raw, rot)
                return go

            def rope_mul(half):
                def go():
                    raw, rot = shared[("rr", half)]
                    cs = slice(half * 1024, (half + 1) * 1024)
                    nc.vector.tensor_mul(raw[:], raw[:], cos_sb[:, cs])
                    nc.vector.tensor_mul(rot[:], rot[:], sin_sb[:, cs])
                    nc.vector.tensor_add(dst_sb[half][:], raw[:], rot[:])
                return go

            def evict_v(half):
                def go():
                    pp = shared[("pp", half)]
                    if "vh" not in shared:
                        shared["vh"] = raws.tile(
                            [P, L], BF, name="vhs", tag="vraw", bufs=2)
                    vh = shared["vh"]
                    nc.vector.tensor_scalar_add(
                        vh[:, half * 1024:(half + 1) * 1024], pp[:], bias_sb[:])
                    if half == 1:
                        vh_cb(vh)
                return go

            if preloaded is None:
                units.append((4, load(0)))
                units.append((4, load(1)))
            for half in range(2):
                units.append((0, alloc_pp(half)))
                for kc in range(8):
                    units.append((2, mm_kc(half, kc)))
                if dst_sb is not None:
                    units.append((2, evict_rot(half)))
                    units.append((2, rope_mul(half)))
                else:
                    units.append((2, evict_v(half)))
            return units

        def vaug_transpose(b):
            # HW xbar transpose requires a contiguous destination; land in
            # per-head vt tiles then DVE-copy into the 65-stride vaug slots.
            def go(vh):
                for h in range(2):
                    vt = rots.tile([P, 1024], BF, name="vt", tag=f"vt{h}",
                                   bufs=2)
                    nc.sync.dma_start_transpose(
                        vt.rearrange("p (t u) -> p t u", u=64),
                        vh[h * DK:(h + 1) * DK, :])
                    va = vaug[b][h].rearrange(
                        "p (t u) -> p t u", u=65)[:, :, 0:64]
                    nc.vector.tensor_copy(
                        va, vt.rearrange("p (t u) -> p t u", u=64))
            return go

        def attention(b, qb, budget=3):
            """512 q tokens; 16 kt tiles; 2 heads row-tiled on the PE."""
            qsl = slice((qb % 2) * 512, (qb % 2) * 512 + 512)
            qq = qq_h[b][qb // 2]
            q0 = qb * 512
            cp = cpp.tile([65, 1024], F32, name="cp", tag="cp")
            ex_prev = None
            for kt in range(NKT + 1):
                ex_cur = None
                if kt < NKT:
                    kk = kk_h[b][kt // 8]
                    ksl = slice((kt % 8) * P, (kt % 8) * P + P)
                    sc = scp.tile([P, 1024], F32, name="sc", tag="sc")
                    nc.tensor.matmul(
                        sc[:, 0:512],
                        lhsT=kk[0:DK, ksl],
                        rhs=qq[0:DK, qsl],
                        start=True, stop=True, skip_group_check=True)
                    nc.tensor.matmul(
                        sc[:, 512:1024],
                        lhsT=kk[DK:P, ksl],
                        rhs=qq[DK:P, qsl],
                        start=True, stop=True, skip_group_check=True)
                    ex_cur = expp.tile([P, 1024], BF, name="ex", tag="ex")
                    nc.scalar.activation(ex_cur[:], sc[:], AF.Exp)
                    if debug_dumps and b == 0 and qb == 0 and kt == 0:
                        nc.sync.dma_start(dbg["dbg_exp"][:], ex_cur[:])
                if kt >= 1:
                    c = kt - 1
                    for h in range(2):
                        nc.tensor.matmul(
                            cp[:, h * 512:(h + 1) * 512],
                            lhsT=vaug[b][h][:, c * 65:(c + 1) * 65],
                            rhs=ex_prev[:, h * 512:(h + 1) * 512],
                            start=(c == 0), stop=(c == NKT - 1),
                            skip_group_check=True)
                ex_prev = ex_cur
                fill(budget)
            # normalize: copy cp out (frees the psum), move den row to
            # partition 0, reciprocal, broadcast, scale both heads.
            cps = smalls.tile([65, 1024], F32, name="cps", tag="cps")
            nc.vector.tensor_copy(cps[:], cp[:])
            den = smalls.tile([1, 1024], F32, name="den", tag="den", bufs=1)
            nc.sync.dma_start(den[:], cps[64:65, :])
            rec = smalls.tile([1, 1024], F32, name="rec", tag="rec", bufs=1)
            nc.vector.reciprocal_approx_fast(rec[:], den[:])
            if debug_dumps and b == 0 and qb == 0:
                nc.sync.dma_start(dbg["dbg_cps"][:], cps[:])
                nc.sync.dma_start(dbg["dbg_rec"][:], rec[:])
            bcs = smalls.tile([DK, 1024], F32, name="bcs", tag="bcs")
            nc.gpsimd.partition_broadcast(bcs[:], rec[:], channels=DK)
            nc.vector.tensor_mul(
                ctx_b[b][0:DK, q0:q0 + 512], cps[0:DK, 0:512], bcs[:, 0:512])
            h1s = h1p.tile([DK, 512], BF, name="h1s", tag="h1s")
            nc.vector.tensor_mul(h1s[:], cps[0:DK, 512:1024], bcs[:, 512:1024])
            nc.sync.dma_start(ctx_b[b][DK:P, q0:q0 + 512], h1s[:])

        def out_units(b, qb, split_evict=False):
            units = []

            def po_unit(tb):
                def go():
                    t0 = qb * 512 + tb * P
                    po = pop.tile([P, D], F32, name="po", tag="pp")
                    for nb in range(2):
                        nc.tensor.matmul(
                            po[:, nb * 512:(nb + 1) * 512],
                            lhsT=ctx_b[b][:, t0:t0 + P],
                            rhs=wo_sb[:, nb * 512:(nb + 1) * 512],
                            start=True, stop=True, skip_group_check=True)
                    ob = outs.tile([P, D], BF, name="ob", tag="ob")
                    row = outp[b * L + t0:b * L + t0 + P, :]
                    if split_evict:
                        for nb in range(2):
                            s = slice(nb * 512, (nb + 1) * 512)
                            nc.vector.tensor_copy(ob[:, s], po[:, s])
                            nc.sync.dma_start(row[:, s], ob[:, s])
                    else:
                        nc.vector.tensor_copy(ob[:], po[:])
                        nc.sync.dma_start(row, ob[:])
                return go

            for tb in range(4):
                units.append((2, po_unit(tb)))
            return units

        # ---------- program ----------
        # prefetch ALL batch-0 inputs up front (transfers stripe across the
        # DMA engines while the projections consume them in arrival order)
        k0_x = (load_half("k", 0, 0), load_half("k", 0, 1))
        wv_sb = load_w("wv_sb", wvH)
        v0_x = (load_half("v", 0, 0), load_half("v", 0, 1))
        wq_sb = load_w("wq_sb", wqH)
        q0_x = (load_half("q", 0, 0), load_half("q", 0, 1))
        wo_sb = const.tile([P, D], BF)
        nc.sync.dma_start(wo_sb[:], woT[:])

        for _, fn in proj_units("k", 0, wk_sb, bk_sb, dst_sb=kk_h[0],
                                preloaded=k0_x):
            fn()
        for _, fn in proj_units("v", 0, wv_sb, bv_sb, vh_cb=vaug_transpose(0),
                                preloaded=v0_x):
            fn()
        for _, fn in proj_units("q", 0, wq_sb, bq_sb, dst_sb=qq_h[0],
                                preloaded=q0_x):
            fn()

        # batch 0 attention; feed batch-1 projections + batch-0 out_proj
        # into the scalar-bound loop as fillers.
        for qb in range(NQB):
            attention(0, qb)
            fillers.extend(out_units(0, qb))
            if qb == 0:
                fillers.extend(proj_units("k", 1, wk_sb, bk_sb, dst_sb=kk_h[1]))
                fillers.extend(proj_units("v", 1, wv_sb, bv_sb,
                                          vh_cb=vaug_transpose(1)))
            elif qb == 1:
                fillers.extend(proj_units("q", 1, wq_sb, bq_sb, dst_sb=qq_h[1]))
        flush()  # b1 attention depends on b1 projections: drain first

        for qb in range(NQB):
            attention(1, qb)
            fillers.extend(out_units(1, qb, split_evict=True))
        flush()

        if debug_dumps:
            for hf in range(2):
                nc.sync.dma_start(
                    dbg["dbg_qq"][:, hf * 1024:(hf + 1) * 1024], qq_h[0][hf][:])
                nc.sync.dma_start(
                    dbg["dbg_kk"][:, hf * 1024:(hf + 1) * 1024], kk_h[0][hf][:])
            nc.sync.dma_start(dbg["dbg_vaug0"][:], vaug[0][0][:])
            nc.sync.dma_start(dbg["dbg_vaug1"][:], vaug[0][1][:])
            nc.sync.dma_start(dbg["dbg_ctx"][:], ctx_b[0][:])

    return nc


def _rope_tables():
    """Host-built RoPE tables [d, t], 2 heads stacked, sign-folded sin."""
    inv_freq = 1.0 / (ROPE_BASE ** (np.arange(0, DK, 2, dtype=np.float64) / DK))
    t = np.arange(L, dtype=np.float64)
    ang = np.outer(t, inv_freq)               # [L, 32]
    emb = np.concatenate([ang, ang], axis=1)  # [L, 64]
    cos = np.cos(emb).T.astype(np.float32)    # [64, L]
    sin = np.sin(emb).T.astype(np.float32)
    sin_folded = sin.copy()
    sin_folded[:32] *= -1.0
    bf = ml_dtypes.bfloat16
    cos2 = np.concatenate([cos, cos], axis=0)                # [128, L]
    sin2 = np.concatenate([sin_folded, sin_folded], axis=0)  # [128, L]
    return cos2.astype(bf), sin2.astype(bf)


def _xh(x):
    """[B, L, D] -> [128, B*2*8*1024]: xH[p, (b, hf, a, t)] = x[b, hf*1024+t,
    a*128+p] — every (b, half) load is per-partition contiguous."""
    bf = ml_dtypes.bfloat16
    xr = np.asarray(x).reshape(B, 2, 1024, 8, P).transpose(4, 0, 1, 3, 2)
    return np.ascontiguousarray(xr.reshape(P, B * 2 * 8 * 1024)).astype(bf)


def _wh(w_slice):
    """[PD, D] weight slice -> [128, 8*128]: wH[p, (a, m)] = W.T[a*128+p, m]."""
    bf = ml_dtypes.bfloat16
    wr = np.ascontiguousarray(w_slice.T).reshape(8, P, PD).transpose(1, 0, 2)
    return np.ascontiguousarray(wr.reshape(P, 8 * PD)).astype(bf)


def host_in_maps(q, k, v, Wq, bq, Wk, bk, Wv, bv, Wo):
    """Per-core input maps (the 1/sqrt(dk) scale is folded into Wq/bq)."""
    bf = ml_dtypes.bfloat16
    qh, kh, vh = _xh(q), _xh(k), _xh(v)
    cos_t, sin_t = _rope_tables()
    scale = 1.0 / np.sqrt(DK)
    in_maps = []
    for c in range(NCORES):
        hs = slice(c * PD, (c + 1) * PD)
        biases = np.stack([
            np.asarray(bq)[hs] * scale, np.asarray(bk)[hs],
            np.asarray(bv)[hs]], axis=1).astype(np.float32)
        in_maps.append({
            "qH": qh, "kH": kh, "vH": vh,
            "wqH": _wh(np.asarray(Wq)[hs, :] * scale),
            "wkH": _wh(np.asarray(Wk)[hs, :]),
            "wvH": _wh(np.asarray(Wv)[hs, :]),
            "woT": np.ascontiguousarray(np.asarray(Wo)[:, hs].T).astype(bf),
            "biases": biases,
            "cos_t": cos_t, "sin_t": sin_t,
        })
    return in_maps


_NC_CACHE = {}


def _get_nc():
    if "nc" not in _NC_CACHE:
        nc = build_nc()
        nc.finalize()
        _NC_CACHE["nc"] = nc
    return _NC_CACHE["nc"]


def kernel(q, k, v, Wq, bq, Wk, bk, Wv, bv, Wo, bo):
    assert q.shape == (B, L, D) and k.shape == (B, L, D) and v.shape == (B, L, D)
    in_maps = host_in_maps(q, k, v, Wq, bq, Wk, bk, Wv, bv, Wo)
    nc = _get_nc()
    res = run_bass_kernel_spmd(nc, in_maps, list(range(NCORES)))
    out = np.zeros((TOK, D), np.float64)
    for r in res.results:
        out += r["outp"].astype(np.float64)
    out += np.asarray(bo, np.float64)[None, :]
    return out.astype(np.float32).reshape(B, L, D)


# revision 40
# speedup vs baseline: 1.4958x; 1.0137x over previous
"""Trainium2 Bass kernel for MultiHeadAttention with RoPE.

Problem: B=2, L=2048, d_model=1024, 16 heads, d_k=64, fp32 in/out.

Sharding (8 cores): tensor-parallel over heads — core c owns heads
{2c, 2c+1}, i.e. a 128-wide slice of the projection output dims.  Every
core reads the full q/k/v activations (transposed + bf16 on host), its
own 128-row slice of Wq/Wk/Wv (pre-transposed; Wq/bq pre-scaled by
1/sqrt(dk)) and the matching 128 columns of Wo.  Each core computes its
heads' attention output and a partial d_model output projection; the
host sums the 8 partials and adds bo.

Per-core pipeline (bf16 matmuls, fp32 PSUM):
  1. QKV projections [128 pd, 1024 tok] halves; bias-add + bf16 evict on
     DVE; RoPE via partition-swap DMAs + 3 DVE ops (sign folded in sin
     table, 1/sqrt(dk) folded into Wq).
  2. V-heads transposed to [kt, dim] layout by ONE dma_start_transpose
     per (batch, head) into a 65-wide-stride "vaug" buffer whose 65th
     column is ones.
  3. Scores: per 128-kt tile, the two heads run CONCURRENTLY on the PE
     as K=64 row-tiles (tile_position (0,0) / (64,0) auto-derived).
  4. exp on ScalarE ([128, 1024] per kt tile covering both heads).
  5. ctx: lhsT = vaug [128 kt, 65] per head; row 64 accumulates the
     softmax denominator for free (M=65 stationary).
  6. normalize: DVE copy of cp, denominator row -> partition 0 via DMA,
     reciprocal + GpSimd broadcast + DVE muls; h1 ctx shifted to
     partitions 64:127 by a small DMA.
  7. out_proj [tok, 1024] = ctx (stationary) @ WoT slice.
Emission interleaves next-batch projections and out_proj matmuls into
the (ScalarE-bound) attention loops as "fillers" so the PE never idles
long and HAM stays at full clock.  All activation buffers are
per-batch tiles so filler writes never create false WAR dependencies
against the running attention.
"""

import collections
import numpy as np
import ml_dtypes

import concourse.bass as bass
import concourse.mybir as mybir
import concourse.tile as tile
from concourse import bacc
from concourse.bass_utils import run_bass_kernel_spmd

BF = mybir.dt.bfloat16
F32 = mybir.dt.float32
AF = mybir.ActivationFunctionType

NCORES = 8
B = 2
L = 2048
D = 1024          # d_model
H = 16            # heads
DK = 64           # head dim
HPC = H // NCORES  # heads per core = 2
PD = HPC * DK      # projection dims per core = 128
TOK = B * L        # 4096 tokens
P = 128
NKT = L // P       # 16 kt tiles per batch
NQB = 4            # 512-token q blocks per batch

ROPE_BASE = 10000.0


def build_nc(debug_dumps=False):
    """Build the single-core Bass program (SPMD: same program, per-core data)."""
    from contextlib import ExitStack

    nc = bacc.Bacc("TRN2", target_bir_lowering=False, debug=False)
    dbg = {}
    if debug_dumps:
        for nm, shp, dt in [
            ("dbg_qq", [P, L], BF), ("dbg_kk", [P, L], BF),
            ("dbg_vaug0", [P, NKT * P], BF), ("dbg_vaug1", [P, NKT * P], BF),
            ("dbg_exp", [P, 1024], BF), ("dbg_cps", [65, 1024], F32),
            ("dbg_rec", [1, 1024], F32), ("dbg_ctx", [P, L], BF),
        ]:
            dbg[nm] = nc.dram_tensor(nm, shp, dt, kind="ExternalOutput").ap()

    # ---- DRAM I/O ----
    # Host pre-arranges everything partition-contiguous so each DMA is 128
    # descriptors of big contiguous chunks (dispatch cost ~ descriptor count).
    # xH[p, (b, half, a, t)]: token t of half `half` of batch b, dim a*128+p.
    qH = nc.dram_tensor("qH", [P, B * 2 * 8 * 1024], BF, kind="ExternalInput").ap()
    kH = nc.dram_tensor("kH", [P, B * 2 * 8 * 1024], BF, kind="ExternalInput").ap()
    vH = nc.dram_tensor("vH", [P, B * 2 * 8 * 1024], BF, kind="ExternalInput").ap()
    # wH[p, (a, m)]: weight row a*128+p, output dim m.
    wqH = nc.dram_tensor("wqH", [P, 8 * P], BF, kind="ExternalInput").ap()
    wkH = nc.dram_tensor("wkH", [P, 8 * P], BF, kind="ExternalInput").ap()
    wvH = nc.dram_tensor("wvH", [P, 8 * P], BF, kind="ExternalInput").ap()
    woT = nc.dram_tensor("woT", [PD, D], BF, kind="ExternalInput").ap()
    bias_d = nc.dram_tensor("biases", [PD, 3], F32, kind="ExternalInput").ap()
    cos_d = nc.dram_tensor("cos_t", [P, L], BF, kind="ExternalInput").ap()
    sin_d = nc.dram_tensor("sin_t", [P, L], BF, kind="ExternalInput").ap()
    outp = nc.dram_tensor("outp", [TOK, D], BF, kind="ExternalOutput").ap()

    xT = {"q": qH, "k": kH, "v": vH}

    with tile.TileContext(nc) as tc, ExitStack() as ctx:
        const = ctx.enter_context(tc.tile_pool(name="const", bufs=1))
        persist = ctx.enter_context(tc.tile_pool(name="persist", bufs=1))
        stage = ctx.enter_context(tc.tile_pool(name="stage", bufs=5))
        raws = ctx.enter_context(tc.tile_pool(name="raws", bufs=2))
        rots = ctx.enter_context(tc.tile_pool(name="rots", bufs=2))
        expp = ctx.enter_context(tc.tile_pool(name="expp", bufs=3))
        outs = ctx.enter_context(tc.tile_pool(name="outs", bufs=3))
        smalls = ctx.enter_context(tc.tile_pool(name="smalls", bufs=2))
        h1p = ctx.enter_context(tc.tile_pool(name="h1p", bufs=2))
        # PSUM: scores 2 tiles x 2 banks + ctx 2 banks + proj/out 2 banks = 8
        scp = ctx.enter_context(tc.tile_pool(name="scp", bufs=2, space="PSUM"))
        cpp = ctx.enter_context(tc.tile_pool(name="cpp", bufs=1, space="PSUM"))
        pop = ctx.enter_context(tc.tile_pool(name="pop", bufs=1, space="PSUM"))

        # ---- constants (emitted in first-use order) ----
        def load_w(name, w_d):
            w_sb = const.tile([P, 8 * P], BF, name=name)
            nc.sync.dma_start(w_sb[:], w_d[:])
            return w_sb

        wk_sb = load_w("wk_sb", wkH)
        bias_sb3 = const.tile([P, 3], F32, name="bias_sb3")
        nc.sync.dma_start(bias_sb3[:], bias_d[:])
        bq_sb = bias_sb3[:, 0:1]
        bk_sb = bias_sb3[:, 1:2]
        bv_sb = bias_sb3[:, 2:3]
        cos_sb = const.tile([P, L], BF)
        nc.sync.dma_start(cos_sb[:], cos_d[:])
        sin_sb = const.tile([P, L], BF)
        nc.sync.dma_start(sin_sb[:], sin_d[:])

        # per-(batch, token-half) persistent activations [128 dims, 1024 tok]
        # (split so scores never wait on the other half's rope chain)
        qq_h = [[persist.tile([P, 1024], BF, name=f"qq{b}_{hf}")
                 for hf in range(2)] for b in range(B)]
        kk_h = [[persist.tile([P, 1024], BF, name=f"kk{b}_{hf}")
                 for hf in range(2)] for b in range(B)]
        ctx_b = [persist.tile([P, L], BF, name=f"ctx{b}") for b in range(B)]
        # vaug[b][h]: 16 slots of [128 kt, 128]; cols 0:64 = v dims
        # (t-major: slot t partition p holds token t*128+p), cols 64:128 all
        # ones — so the ctx matmul replicates the softmax denominator into
        # cp rows 64:128 (no partition broadcast needed to normalize).
        vaug = [[persist.tile([P, NKT * P], BF, name=f"vaug_{b}_{h}")
                 for h in range(2)] for b in range(B)]
        for b in range(B):
            for h in range(2):
                va = vaug[b][h].rearrange("p (t u) -> p t u", u=P)
                nc.vector.memset(va[:, :, DK:P], 1.0)

        # ---------- filler machinery ----------
        fillers = collections.deque()

        def fill(budget):
            while fillers and budget > 0:
                cost, fn = fillers.popleft()
                fn()
                budget -= cost

        def flush():
            while fillers:
                fillers.popleft()[1]()

        # ---------- phase helpers ----------
        def load_half(which, b, half):
            """One 2MB DMA, per-partition contiguous: a 1024-token half."""
            xt = stage.tile([P, 8 * 1024], BF, name="xstage", tag="stage")
            j = (b * 2 + half) * 8192
            nc.sync.dma_start(xt[:], xT[which][:, j:j + 8192])
            return xt

        def proj_units(which, b, w_sb, bias_sb, dst_sb=None, vh_cb=None,
                       preloaded=None, halves=(0, 1), shared=None):
            """Filler units projecting batch b (chosen 1024-token halves).

            dst_sb given -> rope into it (q/k).  vh_cb given -> v path:
            evict to a fresh vh tile, call vh_cb(vh_tile) when done.
            preloaded: stage tiles already loaded.  Pass the same `shared`
            dict when splitting one projection across two calls.
            """
            units = []
            if shared is None:
                shared = {}
            if preloaded is not None:
                shared[("x", 0)], shared[("x", 1)] = preloaded

            def alloc_pp(half):
                def go():
                    shared[("pp", half)] = pop.tile(
                        [P, 1024], F32, name="pp", tag="pp")
                return go

            def mm_kc(half, kc):
                def go():
                    xt = shared[("x", half)].rearrange("p (a t) -> p a t", a=8)
                    pp = shared[("pp", half)]
                    for nb in range(2):
                        nc.tensor.matmul(
                            pp[:, nb * 512:(nb + 1) * 512],
                            lhsT=w_sb[:, kc * P:(kc + 1) * P],
                            rhs=xt[:, kc, nb * 512:(nb + 1) * 512],
                            start=(kc == 0), stop=(kc == 7),
                        )
                return go

            def evict_rot(half):
                def go():
                    pp = shared[("pp", half)]
                    raw = raws.tile([P, 1024], BF, name="raw", tag="raw")
                    nc.vector.tensor_scalar_add(raw[:], pp[:], bias_sb[:])
                    rot = rots.tile([P, 1024], BF, name="rot", tag="rot")
                    for h in range(2):
                        r0 = h * DK
                        nc.sync.dma_start(rot[r0:r0 + 32, :],
                                          raw[r0 + 32:r0 + 64, :])
                        nc.sync.dma_start(rot[r0 + 32:r0 + 64, :],
                                          raw[r0:r0 + 32, :])
                    shared[("rr", half)] = (raw, rot)
                return go

            def rope_mul(half):
                def go():
                    raw, rot = shared[("rr", half)]
                    cs = slice(half * 1024, (half + 1) * 1024)
                    nc.vector.tensor_mul(raw[:], raw[:], cos_sb[:, cs])
                    nc.vector.tensor_mul(rot[:], rot[:], sin_sb[:, cs])
                    nc.vector.tensor_add(dst_sb[half][:], raw[:], rot[:])
                return go

            def evict_v(half):
                def go():
                    pp = shared[("pp", half)]
                    if "vh" not in shared:
                        shared["vh"] = raws.tile(
                            [P, L], BF, name="vhs", tag="vraw", bufs=2)
                    vh = shared["vh"]
                    nc.vector.tensor_scalar_add(
                        vh[:, half * 1024:(half + 1) * 1024], pp[:], bias_sb[:])
                    if half == 1:
                        vh_cb(vh)
                return go

            for half in halves:
                units.append((0, alloc_pp(half)))
                for kc in range(8):
                    units.append((2, mm_kc(half, kc)))
                if dst_sb is not None:
                    units.append((2, evict_rot(half)))
                    units.append((2, rope_mul(half)))
                else:
                    units.append((2, evict_v(half)))
            return units

        def vaug_transpose(b):
            # HW xbar transpose requires a contiguous destination; land in
            # per-head vt tiles then DVE-copy into the 65-stride vaug slots.
            def go(vh):
                for h in range(2):
                    vt = rots.tile([P, 1024], BF, name="vt", tag=f"vt{h}",
                                   bufs=2)
                    nc.sync.dma_start_transpose(
                        vt.rearrange("p (t u) -> p t u", u=64),
                        vh[h * DK:(h + 1) * DK, :])
                    va = vaug[b][h].rearrange(
                        "p (t u) -> p t u", u=P)[:, :, 0:DK]
                    nc.vector.tensor_copy(
                        va, vt.rearrange("p (t u) -> p t u", u=64))
            return go

        def attention(b, qb, budget=3):
            """512 q tokens; 16 kt tiles; 2 heads row-tiled on the PE."""
            qsl = slice((qb % 2) * 512, (qb % 2) * 512 + 512)
            qq = qq_h[b][qb // 2]
            q0 = qb * 512
            cp = cpp.tile([P, 1024], F32, name="cp", tag="cp")
            ex_prev = None
            for kt in range(NKT + 1):
                ex_cur = None
                if kt < NKT:
                    kk = kk_h[b][kt // 8]
                    ksl = slice((kt % 8) * P, (kt % 8) * P + P)
                    sc = scp.tile([P, 1024], F32, name="sc", tag="sc")
                    nc.tensor.matmul(
                        sc[:, 0:512],
                        lhsT=kk[0:DK, ksl],
                        rhs=qq[0:DK, qsl],
                        start=True, stop=True, skip_group_check=True)
                    nc.tensor.matmul(
                        sc[:, 512:1024],
                        lhsT=kk[DK:P, ksl],
                        rhs=qq[DK:P, qsl],
                        start=True, stop=True, skip_group_check=True)
                    ex_cur = expp.tile([P, 1024], BF, name="ex", tag="ex")
                    nc.scalar.activation(ex_cur[:], sc[:], AF.Exp)
                    if debug_dumps and b == 0 and qb == 0 and kt == 0:
                        nc.sync.dma_start(dbg["dbg_exp"][:], ex_cur[:])
                if kt >= 1:
                    c = kt - 1
                    for h in range(2):
                        nc.tensor.matmul(
                            cp[:, h * 512:(h + 1) * 512],
                            lhsT=vaug[b][h][:, c * P:(c + 1) * P],
                            rhs=ex_prev[:, h * 512:(h + 1) * 512],
                            start=(c == 0), stop=(c == NKT - 1),
                            skip_group_check=True)
                ex_prev = ex_cur
                fill(budget)
            # normalize: cp rows 64:128 all hold the denominator; copy cp
            # out (frees the psum), DMA the denominator rows to partition 0,
            # reciprocal, scale both heads.
            cps = smalls.tile([P, 1024], F32, name="cps", tag="cps")
            nc.vector.tensor_copy(cps[:], cp[:])
            den = smalls.tile([DK, 1024], F32, name="den", tag="den", bufs=1)
            nc.sync.dma_start(den[:], cps[DK:P, :])
            rec = smalls.tile([DK, 1024], F32, name="rec", tag="rec", bufs=1)
            nc.vector.reciprocal_approx_fast(rec[:], den[:])
            if debug_dumps and b == 0 and qb == 0:
                nc.sync.dma_start(dbg["dbg_cps"][:], cps[0:65, :])
                nc.sync.dma_start(dbg["dbg_rec"][:], rec[0:1, :])
            nc.vector.tensor_mul(
                ctx_b[b][0:DK, q0:q0 + 512], cps[0:DK, 0:512], rec[:, 0:512])
            h1s = h1p.tile([DK, 512], BF, name="h1s", tag="h1s")
            nc.vector.tensor_mul(h1s[:], cps[0:DK, 512:1024], rec[:, 512:1024])
            nc.sync.dma_start(ctx_b[b][DK:P, q0:q0 + 512], h1s[:])

        def out_units(b, qb, tail=False):
            units = []

            def po_unit(tb):
                def go():
                    t0 = qb * 512 + tb * P
                    # in the end-of-kernel tail, alternate PSUM pools so
                    # consecutive units pipeline instead of serializing on
                    # the single pop buffer (scores pool is idle by then)
                    if tail and tb % 2:
                        po = scp.tile([P, D], F32, name="sc", tag="sc")
                    else:
                        po = pop.tile([P, D], F32, name="po", tag="pp")
                    for nb in range(2):
                        nc.tensor.matmul(
                            po[:, nb * 512:(nb + 1) * 512],
                            lhsT=ctx_b[b][:, t0:t0 + P],
                            rhs=wo_sb[:, nb * 512:(nb + 1) * 512],
                            start=True, stop=True, skip_group_check=True)
                    ob = outs.tile([P, D], BF, name="ob", tag="ob")
                    nc.vector.tensor_copy(ob[:], po[:])
                    nc.sync.dma_start(outp[b * L + t0:b * L + t0 + P, :], ob[:])
                return go

            for tb in range(4):
                units.append((2, po_unit(tb)))
            return units

        # ---------- program ----------
        # Batch-0 inputs prefetched in consumption-criticality order:
        # v first (the vaug chain gates the first ctx matmul), then the
        # h0 halves of k and q (which gate the first scores), then k-h1 /
        # q-h1 whose projections run as the first attention fillers.
        v0_x = (load_half("v", 0, 0), load_half("v", 0, 1))
        wv_sb = load_w("wv_sb", wvH)
        wq_sb = load_w("wq_sb", wqH)
        k0_x = (load_half("k", 0, 0), None)
        q0_x = (load_half("q", 0, 0), None)
        k0_x = (k0_x[0], load_half("k", 0, 1))
        q0_x = (q0_x[0], load_half("q", 0, 1))
        wo_sb = const.tile([P, D], BF)
        nc.sync.dma_start(wo_sb[:], woT[:])

        for _, fn in proj_units("v", 0, wv_sb, bv_sb, vh_cb=vaug_transpose(0),
                                preloaded=v0_x):
            fn()
        ksh, qsh = {}, {}
        for _, fn in proj_units("k", 0, wk_sb, bk_sb, dst_sb=kk_h[0],
                                preloaded=k0_x, halves=(0,), shared=ksh):
            fn()
        for _, fn in proj_units("q", 0, wq_sb, bq_sb, dst_sb=qq_h[0],
                                preloaded=q0_x, halves=(0,), shared=qsh):
            fn()
        # second halves become the first attention fillers
        fillers.extend(proj_units("k", 0, wk_sb, bk_sb, dst_sb=kk_h[0],
                                  halves=(1,), shared=ksh))
        fillers.extend(proj_units("q", 0, wq_sb, bq_sb, dst_sb=qq_h[0],
                                  halves=(1,), shared=qsh))

        # batch 0 attention; feed batch-1 projections + batch-0 out_proj
        # into the scalar-bound loop as fillers (batch-1 input DMAs are
        # dispatched inline at block boundaries for transfer lead time).
        for qb in range(NQB):
            attention(0, qb)
            fillers.extend(out_units(0, qb))
            if qb == 0:
                k1_x = (load_half("k", 1, 0), load_half("k", 1, 1))
                v1_x = (load_half("v", 1, 0), load_half("v", 1, 1))
                fillers.extend(proj_units("k", 1, wk_sb, bk_sb,
                                          dst_sb=kk_h[1], preloaded=k1_x))
                fillers.extend(proj_units("v", 1, wv_sb, bv_sb,
                                          vh_cb=vaug_transpose(1),
                                          preloaded=v1_x))
            elif qb == 1:
                q1_x = (load_half("q", 1, 0), load_half("q", 1, 1))
                fillers.extend(proj_units("q", 1, wq_sb, bq_sb,
                                          dst_sb=qq_h[1], preloaded=q1_x))
        flush()  # b1 attention depends on b1 projections: drain first

        for qb in range(NQB):
            attention(1, qb)
            fillers.extend(out_units(1, qb, tail=(qb == NQB - 1)))
        flush()

        if debug_dumps:
            for hf in range(2):
                nc.sync.dma_start(
                    dbg["dbg_qq"][:, hf * 1024:(hf + 1) * 1024], qq_h[0][hf][:])
                nc.sync.dma_start(
                    dbg["dbg_kk"][:, hf * 1024:(hf + 1) * 1024], kk_h[0][hf][:])
            nc.sync.dma_start(dbg["dbg_vaug0"][:], vaug[0][0][:])
            nc.sync.dma_start(dbg["dbg_vaug1"][:], vaug[0][1][:])
            nc.sync.dma_start(dbg["dbg_ctx"][:], ctx_b[0][:])

    return nc


def _rope_tables():
    """Host-built RoPE tables [d, t], 2 heads stacked, sign-folded sin."""
    inv_freq = 1.0 / (ROPE_BASE ** (np.arange(0, DK, 2, dtype=np.float64) / DK))
    t = np.arange(L, dtype=np.float64)
    ang = np.outer(t, inv_freq)               # [L, 32]
    emb = np.concatenate([ang, ang], axis=1)  # [L, 64]
    cos = np.cos(emb).T.astype(np.float32)    # [64, L]
    sin = np.sin(emb).T.astype(np.float32)
    sin_folded = sin.copy()
    sin_folded[:32] *= -1.0
    bf = ml_dtypes.bfloat16
    cos2 = np.concatenate([cos, cos], axis=0)                # [128, L]
    sin2 = np.concatenate([sin_folded, sin_folded], axis=0)  # [128, L]
    return cos2.astype(bf), sin2.astype(bf)


def _xh(x):
    """[B, L, D] -> [128, B*2*8*1024]: xH[p, (b, hf, a, t)] = x[b, hf*1024+t,
    a*128+p] — every (b, half) load is per-partition contiguous."""
    bf = ml_dtypes.bfloat16
    xr = np.asarray(x).reshape(B, 2, 1024, 8, P).transpose(4, 0, 1, 3, 2)
    return np.ascontiguousarray(xr.reshape(P, B * 2 * 8 * 1024)).astype(bf)


def _wh(w_slice):
    """[PD, D] weight slice -> [128, 8*128]: wH[p, (a, m)] = W.T[a*128+p, m]."""
    bf = ml_dtypes.bfloat16
    wr = np.ascontiguousarray(w_slice.T).reshape(8, P, PD).transpose(1, 0, 2)
    return np.ascontiguousarray(wr.reshape(P, 8 * PD)).astype(bf)


def host_in_maps(q, k, v, Wq, bq, Wk, bk, Wv, bv, Wo):
    """Per-core input maps (the 1/sqrt(dk) scale is folded into Wq/bq)."""
    bf = ml_dtypes.bfloat16
    qh, kh, vh = _xh(q), _xh(k), _xh(v)
    cos_t, sin_t = _rope_tables()
    scale = 1.0 / np.sqrt(DK)
    in_maps = []
    for c in range(NCORES):
        hs = slice(c * PD, (c + 1) * PD)
        biases = np.stack([
            np.asarray(bq)[hs] * scale, np.asarray(bk)[hs],
            np.asarray(bv)[hs]], axis=1).astype(np.float32)
        in_maps.append({
            "qH": qh, "kH": kh, "vH": vh,
            "wqH": _wh(np.asarray(Wq)[hs, :] * scale),
            "wkH": _wh(np.asarray(Wk)[hs, :]),
            "wvH": _wh(np.asarray(Wv)[hs, :]),
            "woT": np.ascontiguousarray(np.asarray(Wo)[:, hs].T).astype(bf),
            "biases": biases,
            "cos_t": cos_t, "sin_t": sin_t,
        })
    return in_maps


_NC_CACHE = {}


def _get_nc():
    if "nc" not in _NC_CACHE:
        nc = build_nc()
        nc.finalize()
        _NC_CACHE["nc"] = nc
    return _NC_CACHE["nc"]


def kernel(q, k, v, Wq, bq, Wk, bk, Wv, bv, Wo, bo):
    assert q.shape == (B, L, D) and k.shape == (B, L, D) and v.shape == (B, L, D)
    in_maps = host_in_maps(q, k, v, Wq, bq, Wk, bk, Wv, bv, Wo)
    nc = _get_nc()
    res = run_bass_kernel_spmd(nc, in_maps, list(range(NCORES)))
    out = np.zeros((TOK, D), np.float64)
    for r in res.results:
        out += r["outp"].astype(np.float64)
    out += np.asarray(bo, np.float64)[None, :]
    return out.astype(np.float32).reshape(B, L, D)


# revision 43
# speedup vs baseline: 1.4973x; 1.0010x over previous
"""Trainium2 Bass kernel for MultiHeadAttention with RoPE.

Problem: B=2, L=2048, d_model=1024, 16 heads, d_k=64, fp32 in/out.

Sharding (8 cores): tensor-parallel over heads — core c owns heads
{2c, 2c+1}, i.e. a 128-wide slice of the projection output dims.  Every
core reads the full q/k/v activations (transposed + bf16 on host), its
own 128-row slice of Wq/Wk/Wv (pre-transposed; Wq/bq pre-scaled by
1/sqrt(dk)) and the matching 128 columns of Wo.  Each core computes its
heads' attention output and a partial d_model output projection; the
host sums the 8 partials and adds bo.

Per-core pipeline (bf16 matmuls, fp32 PSUM):
  1. QKV projections [128 pd, 1024 tok] halves; bias-add + bf16 evict on
     DVE; RoPE via partition-swap DMAs + 3 DVE ops (sign folded in sin
     table, 1/sqrt(dk) folded into Wq).
  2. V-heads transposed to [kt, dim] layout by ONE dma_start_transpose
     per (batch, head) into a 65-wide-stride "vaug" buffer whose 65th
     column is ones.
  3. Scores: per 128-kt tile, the two heads run CONCURRENTLY on the PE
     as K=64 row-tiles (tile_position (0,0) / (64,0) auto-derived).
  4. exp on ScalarE ([128, 1024] per kt tile covering both heads).
  5. ctx: lhsT = vaug [128 kt, 65] per head; row 64 accumulates the
     softmax denominator for free (M=65 stationary).
  6. normalize: DVE copy of cp, denominator row -> partition 0 via DMA,
     reciprocal + GpSimd broadcast + DVE muls; h1 ctx shifted to
     partitions 64:127 by a small DMA.
  7. out_proj [tok, 1024] = ctx (stationary) @ WoT slice.
Emission interleaves next-batch projections and out_proj matmuls into
the (ScalarE-bound) attention loops as "fillers" so the PE never idles
long and HAM stays at full clock.  All activation buffers are
per-batch tiles so filler writes never create false WAR dependencies
against the running attention.
"""

import collections
import numpy as np
import ml_dtypes

import concourse.bass as bass
import concourse.mybir as mybir
import concourse.tile as tile
from concourse import bacc
from concourse.bass_utils import run_bass_kernel_spmd

BF = mybir.dt.bfloat16
F32 = mybir.dt.float32
AF = mybir.ActivationFunctionType

NCORES = 8
B = 2
L = 2048
D = 1024          # d_model
H = 16            # heads
DK = 64           # head dim
HPC = H // NCORES  # heads per core = 2
PD = HPC * DK      # projection dims per core = 128
TOK = B * L        # 4096 tokens
P = 128
NKT = L // P       # 16 kt tiles per batch
NQB = 4            # 512-token q blocks per batch

ROPE_BASE = 10000.0


def build_nc(debug_dumps=False):
    """Build the single-core Bass program (SPMD: same program, per-core data)."""
    from contextlib import ExitStack

    nc = bacc.Bacc("TRN2", target_bir_lowering=False, debug=False)
    dbg = {}
    if debug_dumps:
        for nm, shp, dt in [
            ("dbg_qq", [P, L], BF), ("dbg_kk", [P, L], BF),
            ("dbg_vaug0", [P, NKT * P], BF), ("dbg_vaug1", [P, NKT * P], BF),
            ("dbg_exp", [P, 1024], BF), ("dbg_cps", [65, 1024], F32),
            ("dbg_rec", [1, 1024], F32), ("dbg_ctx", [P, L], BF),
        ]:
            dbg[nm] = nc.dram_tensor(nm, shp, dt, kind="ExternalOutput").ap()

    # ---- DRAM I/O ----
    # Host pre-arranges everything partition-contiguous so each DMA is 128
    # descriptors of big contiguous chunks (dispatch cost ~ descriptor count).
    # xH[p, (b, half, a, t)]: token t of half `half` of batch b, dim a*128+p.
    qH = nc.dram_tensor("qH", [P, B * 2 * 8 * 1024], BF, kind="ExternalInput").ap()
    kH = nc.dram_tensor("kH", [P, B * 2 * 8 * 1024], BF, kind="ExternalInput").ap()
    vH = nc.dram_tensor("vH", [P, B * 2 * 8 * 1024], BF, kind="ExternalInput").ap()
    # wH[p, (a, m)]: weight row a*128+p, output dim m.
    wqH = nc.dram_tensor("wqH", [P, 8 * P], BF, kind="ExternalInput").ap()
    wkH = nc.dram_tensor("wkH", [P, 8 * P], BF, kind="ExternalInput").ap()
    wvH = nc.dram_tensor("wvH", [P, 8 * P], BF, kind="ExternalInput").ap()
    woT = nc.dram_tensor("woT", [PD, D], BF, kind="ExternalInput").ap()
    bias_d = nc.dram_tensor("biases", [PD, 3], F32, kind="ExternalInput").ap()
    cos_d = nc.dram_tensor("cos_t", [P, L], BF, kind="ExternalInput").ap()
    sin_d = nc.dram_tensor("sin_t", [P, L], BF, kind="ExternalInput").ap()
    outp = nc.dram_tensor("outp", [TOK, D], BF, kind="ExternalOutput").ap()

    xT = {"q": qH, "k": kH, "v": vH}

    with tile.TileContext(nc) as tc, ExitStack() as ctx:
        const = ctx.enter_context(tc.tile_pool(name="const", bufs=1))
        persist = ctx.enter_context(tc.tile_pool(name="persist", bufs=1))
        stage = ctx.enter_context(tc.tile_pool(name="stage", bufs=5))
        raws = ctx.enter_context(tc.tile_pool(name="raws", bufs=2))
        rots = ctx.enter_context(tc.tile_pool(name="rots", bufs=2))
        expp = ctx.enter_context(tc.tile_pool(name="expp", bufs=3))
        outs = ctx.enter_context(tc.tile_pool(name="outs", bufs=3))
        smalls = ctx.enter_context(tc.tile_pool(name="smalls", bufs=2))
        h1p = ctx.enter_context(tc.tile_pool(name="h1p", bufs=2))
        # PSUM: scores 2 tiles x 2 banks + ctx 2 banks + proj/out 2 banks = 8
        scp = ctx.enter_context(tc.tile_pool(name="scp", bufs=2, space="PSUM"))
        cpp = ctx.enter_context(tc.tile_pool(name="cpp", bufs=1, space="PSUM"))
        pop = ctx.enter_context(tc.tile_pool(name="pop", bufs=1, space="PSUM"))

        # ---- constants (emitted in first-use order) ----
        def load_w(name, w_d):
            w_sb = const.tile([P, 8 * P], BF, name=name)
            nc.sync.dma_start(w_sb[:], w_d[:])
            return w_sb

        wk_sb = load_w("wk_sb", wkH)
        bias_sb3 = const.tile([P, 3], F32, name="bias_sb3")
        nc.sync.dma_start(bias_sb3[:], bias_d[:])
        bq_sb = bias_sb3[:, 0:1]
        bk_sb = bias_sb3[:, 1:2]
        bv_sb = bias_sb3[:, 2:3]
        cos_sb = const.tile([P, L], BF)
        nc.sync.dma_start(cos_sb[:], cos_d[:])
        sin_sb = const.tile([P, L], BF)
        nc.sync.dma_start(sin_sb[:], sin_d[:])

        # per-(batch, token-half) persistent activations [128 dims, 1024 tok]
        # (split so scores never wait on the other half's rope chain)
        qq_h = [[persist.tile([P, 1024], BF, name=f"qq{b}_{hf}")
                 for hf in range(2)] for b in range(B)]
        kk_h = [[persist.tile([P, 1024], BF, name=f"kk{b}_{hf}")
                 for hf in range(2)] for b in range(B)]
        ctx_b = [persist.tile([P, L], BF, name=f"ctx{b}") for b in range(B)]
        # vaug[b][h]: 16 slots of [128 kt, 128]; cols 0:64 = v dims
        # (t-major: slot t partition p holds token t*128+p), cols 64:128 all
        # ones — so the ctx matmul replicates the softmax denominator into
        # cp rows 64:128 (no partition broadcast needed to normalize).
        vaug = [[persist.tile([P, NKT * P], BF, name=f"vaug_{b}_{h}")
                 for h in range(2)] for b in range(B)]
        for b in range(B):
            for h in range(2):
                va = vaug[b][h].rearrange("p (t u) -> p t u", u=P)
                nc.vector.memset(va[:, :, DK:P], 1.0)

        # ---------- filler machinery ----------
        fillers = collections.deque()

        def fill(budget):
            while fillers and budget > 0:
                cost, fn = fillers.popleft()
                fn()
                budget -= cost

        def flush():
            while fillers:
                fillers.popleft()[1]()

        # ---------- phase helpers ----------
        def load_half(which, b, half):
            """One 2MB DMA, per-partition contiguous: a 1024-token half."""
            xt = stage.tile([P, 8 * 1024], BF, name="xstage", tag="stage")
            j = (b * 2 + half) * 8192
            nc.sync.dma_start(xt[:], xT[which][:, j:j + 8192])
            return xt

        def proj_units(which, b, w_sb, bias_sb, dst_sb=None, vh_cb=None,
                       preloaded=None, halves=(0, 1), shared=None):
            """Filler units projecting batch b (chosen 1024-token halves).

            dst_sb given -> rope into it (q/k).  vh_cb given -> v path:
            evict to a fresh vh tile, call vh_cb(vh_tile) when done.
            preloaded: stage tiles already loaded.  Pass the same `shared`
            dict when splitting one projection across two calls.
            """
            units = []
            if shared is None:
                shared = {}
            if preloaded is not None:
                shared[("x", 0)], shared[("x", 1)] = preloaded

            def alloc_pp(half):
                def go():
                    shared[("pp", half)] = pop.tile(
                        [P, 1024], F32, name="pp", tag="pp")
                return go

            def mm_kc(half, kc):
                def go():
                    xt = shared[("x", half)].rearrange("p (a t) -> p a t", a=8)
                    pp = shared[("pp", half)]
                    for nb in range(2):
                        nc.tensor.matmul(
                            pp[:, nb * 512:(nb + 1) * 512],
                            lhsT=w_sb[:, kc * P:(kc + 1) * P],
                            rhs=xt[:, kc, nb * 512:(nb + 1) * 512],
                            start=(kc == 0), stop=(kc == 7),
                        )
                return go

            def evict_rot(half):
                def go():
                    pp = shared[("pp", half)]
                    raw = raws.tile([P, 1024], BF, name="raw", tag="raw")
                    nc.vector.tensor_scalar_add(raw[:], pp[:], bias_sb[:])
                    rot = rots.tile([P, 1024], BF, name="rot", tag="rot")
                    for h in range(2):
                        r0 = h * DK
                        nc.sync.dma_start(rot[r0:r0 + 32, :],
                                          raw[r0 + 32:r0 + 64, :])
                        nc.sync.dma_start(rot[r0 + 32:r0 + 64, :],
                                          raw[r0:r0 + 32, :])
                    shared[("rr", half)] = (raw, rot)
                return go

            def rope_mul(half):
                def go():
                    raw, rot = shared[("rr", half)]
                    cs = slice(half * 1024, (half + 1) * 1024)
                    nc.vector.tensor_mul(raw[:], raw[:], cos_sb[:, cs])
                    nc.vector.tensor_mul(rot[:], rot[:], sin_sb[:, cs])
                    nc.vector.tensor_add(dst_sb[half][:], raw[:], rot[:])
                return go

            def evict_v(half):
                def go():
                    pp = shared[("pp", half)]
                    if "vh" not in shared:
                        shared["vh"] = raws.tile(
                            [P, L], BF, name="vhs", tag="vraw", bufs=2)
                    vh = shared["vh"]
                    nc.vector.tensor_scalar_add(
                        vh[:, half * 1024:(half + 1) * 1024], pp[:], bias_sb[:])
                    if half == 1:
                        vh_cb(vh)
                return go

            for half in halves:
                units.append((0, alloc_pp(half)))
                for kc in range(8):
                    units.append((2, mm_kc(half, kc)))
                if dst_sb is not None:
                    units.append((2, evict_rot(half)))
                    units.append((2, rope_mul(half)))
                else:
                    units.append((2, evict_v(half)))
            return units

        def vaug_transpose(b):
            # HW xbar transpose requires a contiguous destination; land in
            # per-head vt tiles then DVE-copy into the 65-stride vaug slots.
            def go(vh):
                for h in range(2):
                    vt = rots.tile([P, 1024], BF, name="vt", tag=f"vt{h}",
                                   bufs=2)
                    nc.sync.dma_start_transpose(
                        vt.rearrange("p (t u) -> p t u", u=64),
                        vh[h * DK:(h + 1) * DK, :])
                    va = vaug[b][h].rearrange(
                        "p (t u) -> p t u", u=P)[:, :, 0:DK]
                    nc.vector.tensor_copy(
                        va, vt.rearrange("p (t u) -> p t u", u=64))
            return go

        def attention(b, qb, budget=3):
            """512 q tokens; 16 kt tiles; 2 heads row-tiled on the PE."""
            qsl = slice((qb % 2) * 512, (qb % 2) * 512 + 512)
            qq = qq_h[b][qb // 2]
            q0 = qb * 512
            cp = cpp.tile([P, 1024], F32, name="cp", tag="cp")
            ex_prev = None
            for kt in range(NKT + 1):
                ex_cur = None
                if kt < NKT:
                    kk = kk_h[b][kt // 8]
                    ksl = slice((kt % 8) * P, (kt % 8) * P + P)
                    sc = scp.tile([P, 1024], F32, name="sc", tag="sc")
                    nc.tensor.matmul(
                        sc[:, 0:512],
                        lhsT=kk[0:DK, ksl],
                        rhs=qq[0:DK, qsl],
                        start=True, stop=True, skip_group_check=True)
                    nc.tensor.matmul(
                        sc[:, 512:1024],
                        lhsT=kk[DK:P, ksl],
                        rhs=qq[DK:P, qsl],
                        start=True, stop=True, skip_group_check=True)
                    ex_cur = expp.tile([P, 1024], BF, name="ex", tag="ex")
                    nc.scalar.activation(ex_cur[:], sc[:], AF.Exp)
                    if debug_dumps and b == 0 and qb == 0 and kt == 0:
                        nc.sync.dma_start(dbg["dbg_exp"][:], ex_cur[:])
                if kt >= 1:
                    c = kt - 1
                    for h in range(2):
                        nc.tensor.matmul(
                            cp[:, h * 512:(h + 1) * 512],
                            lhsT=vaug[b][h][:, c * P:(c + 1) * P],
                            rhs=ex_prev[:, h * 512:(h + 1) * 512],
                            start=(c == 0), stop=(c == NKT - 1),
                            skip_group_check=True)
                ex_prev = ex_cur
                fill(budget)
            # normalize: cp rows 64:128 all hold the denominator; copy cp
            # out (frees the psum), DMA the denominator rows to partition 0,
            # reciprocal, scale both heads.
            cps = smalls.tile([P, 1024], F32, name="cps", tag="cps")
            nc.vector.tensor_copy(cps[:], cp[:])
            den = smalls.tile([DK, 1024], F32, name="den", tag="den", bufs=1)
            nc.sync.dma_start(den[:], cps[DK:P, :])
            rec = smalls.tile([DK, 1024], F32, name="rec", tag="rec", bufs=1)
            nc.vector.reciprocal_approx_fast(rec[:], den[:])
            if debug_dumps and b == 0 and qb == 0:
                nc.sync.dma_start(dbg["dbg_cps"][:], cps[0:65, :])
                nc.sync.dma_start(dbg["dbg_rec"][:], rec[0:1, :])
            nc.vector.tensor_mul(
                ctx_b[b][0:DK, q0:q0 + 512], cps[0:DK, 0:512], rec[:, 0:512])
            h1s = h1p.tile([DK, 512], BF, name="h1s", tag="h1s")
            nc.vector.tensor_mul(h1s[:], cps[0:DK, 512:1024], rec[:, 512:1024])
            nc.sync.dma_start(ctx_b[b][DK:P, q0:q0 + 512], h1s[:])

        def out_units(b, qb, tail=False):
            units = []

            def po_unit(tb):
                def go():
                    t0 = qb * 512 + tb * P
                    # in the end-of-kernel tail, alternate PSUM pools so
                    # consecutive units pipeline instead of serializing on
                    # the single pop buffer (scores pool is idle by then)
                    if tail and tb % 2:
                        po = scp.tile([P, D], F32, name="sc", tag="sc")
                    else:
                        po = pop.tile([P, D], F32, name="po", tag="pp")
                    for nb in range(2):
                        nc.tensor.matmul(
                            po[:, nb * 512:(nb + 1) * 512],
                            lhsT=ctx_b[b][:, t0:t0 + P],
                            rhs=wo_sb[:, nb * 512:(nb + 1) * 512],
                            start=True, stop=True, skip_group_check=True)
                    ob = outs.tile([P, D], BF, name="ob", tag="ob")
                    nc.vector.tensor_copy(ob[:], po[:])
                    nc.sync.dma_start(outp[b * L + t0:b * L + t0 + P, :], ob[:])
                return go

            for tb in range(4):
                units.append((2, po_unit(tb)))
            return units

        def load_bar(tile):
            """Tiny DMA reading `tile`: holds the sync queue until tile's
            load transfer completes, serializing big transfers so they
            finish in consumption order instead of round-robin sharing."""
            dm = smalls.tile([1, 2], BF, name="bar", tag="bar", bufs=2)
            nc.sync.dma_start(dm[:], tile[0:1, 0:2])

        # ---------- program ----------
        # Batch-0 inputs prefetched in consumption-criticality order and
        # SERIALIZED via load barriers: v first (the vaug chain gates the
        # first ctx matmul), then the h0 halves of k and q (which gate the
        # first scores), then k-h1 / q-h1 for the filler projections.
        wv_sb = load_w("wv_sb", wvH)
        wq_sb = load_w("wq_sb", wqH)
        v0a = load_half("v", 0, 0)
        v0b = load_half("v", 0, 1)
        v0_x = (v0a, v0b)
        load_bar(v0a)
        k0a = load_half("k", 0, 0)
        wo_sb = const.tile([P, D], BF)
        nc.sync.dma_start(wo_sb[:], woT[:])
        load_bar(v0b)
        q0a = load_half("q", 0, 0)
        load_bar(k0a)
        k0b = load_half("k", 0, 1)
        load_bar(q0a)
        q0b = load_half("q", 0, 1)
        k0_x = (k0a, k0b)
        q0_x = (q0a, q0b)

        for _, fn in proj_units("v", 0, wv_sb, bv_sb, vh_cb=vaug_transpose(0),
                                preloaded=v0_x):
            fn()
        ksh, qsh = {}, {}
        for _, fn in proj_units("k", 0, wk_sb, bk_sb, dst_sb=kk_h[0],
                                preloaded=k0_x, halves=(0,), shared=ksh):
            fn()
        for _, fn in proj_units("q", 0, wq_sb, bq_sb, dst_sb=qq_h[0],
                                preloaded=q0_x, halves=(0,), shared=qsh):
            fn()
        # second halves become the first attention fillers
        fillers.extend(proj_units("k", 0, wk_sb, bk_sb, dst_sb=kk_h[0],
                                  halves=(1,), shared=ksh))
        fillers.extend(proj_units("q", 0, wq_sb, bq_sb, dst_sb=qq_h[0],
                                  halves=(1,), shared=qsh))
        # batch-1 k prefetch rides the tail of the batch-0 load stream
        xk1 = (load_half("k", 1, 0), load_half("k", 1, 1))

        # batch 0 attention; feed batch-1 projections + batch-0 out_proj
        # into the scalar-bound loop as fillers (batch-1 input DMAs are
        # dispatched inline at block boundaries for transfer lead time).
        for qb in range(NQB):
            attention(0, qb)
            fillers.extend(out_units(0, qb))
            if qb == 0:
                xv1 = (load_half("v", 1, 0), load_half("v", 1, 1))
                fillers.extend(proj_units("k", 1, wk_sb, bk_sb,
                                          dst_sb=kk_h[1], preloaded=xk1))
                fillers.extend(proj_units("v", 1, wv_sb, bv_sb,
                                          vh_cb=vaug_transpose(1),
                                          preloaded=xv1))
            elif qb == 1:
                xq1 = (load_half("q", 1, 0), load_half("q", 1, 1))
                fillers.extend(proj_units("q", 1, wq_sb, bq_sb,
                                          dst_sb=qq_h[1], preloaded=xq1))
        flush()  # b1 attention depends on b1 projections: drain first

        for qb in range(NQB):
            attention(1, qb)
            fillers.extend(out_units(1, qb, tail=(qb == NQB - 1)))
        flush()

        if debug_dumps:
            for hf in range(2):
                nc.sync.dma_start(
                    dbg["dbg_qq"][:, hf * 1024:(hf + 1) * 1024], qq_h[0][hf][:])
                nc.sync.dma_start(
                    dbg["dbg_kk"][:, hf * 1024:(hf + 1) * 1024], kk_h[0][hf][:])
            nc.sync.dma_start(dbg["dbg_vaug0"][:], vaug[0][0][:])
            nc.sync.dma_start(dbg["dbg_vaug1"][:], vaug[0][1][:])
            nc.sync.dma_start(dbg["dbg_ctx"][:], ctx_b[0][:])

    return nc


def _rope_tables():
    """Host-built RoPE tables [d, t], 2 heads stacked, sign-folded sin."""
    inv_freq = 1.0 / (ROPE_BASE ** (np.arange(0, DK, 2, dtype=np.float64) / DK))
    t = np.arange(L, dtype=np.float64)
    ang = np.outer(t, inv_freq)               # [L, 32]
    emb = np.concatenate([ang, ang], axis=1)  # [L, 64]
    cos = np.cos(emb).T.astype(np.float32)    # [64, L]
    sin = np.sin(emb).T.astype(np.float32)
    sin_folded = sin.copy()
    sin_folded[:32] *= -1.0
    bf = ml_dtypes.bfloat16
    cos2 = np.concatenate([cos, cos], axis=0)                # [128, L]
    sin2 = np.concatenate([sin_folded, sin_folded], axis=0)  # [128, L]
    return cos2.astype(bf), sin2.astype(bf)


def _xh(x):
    """[B, L, D] -> [128, B*2*8*1024]: xH[p, (b, hf, a, t)] = x[b, hf*1024+t,
    a*128+p] — every (b, half) load is per-partition contiguous."""
    bf = ml_dtypes.bfloat16
    xr = np.asarray(x).reshape(B, 2, 1024, 8, P).transpose(4, 0, 1, 3, 2)
    return np.ascontiguousarray(xr.reshape(P, B * 2 * 8 * 1024)).astype(bf)


def _wh(w_slice):
    """[PD, D] weight slice -> [128, 8*128]: wH[p, (a, m)] = W.T[a*128+p, m]."""
    bf = ml_dtypes.bfloat16
    wr = np.ascontiguousarray(w_slice.T).reshape(8, P, PD).transpose(1, 0, 2)
    return np.ascontiguousarray(wr.reshape(P, 8 * PD)).astype(bf)


def host_in_maps(q, k, v, Wq, bq, Wk, bk, Wv, bv, Wo):
    """Per-core input maps (the 1/sqrt(dk) scale is folded into Wq/bq)."""
    bf = ml_dtypes.bfloat16
    qh, kh, vh = _xh(q), _xh(k), _xh(v)
    cos_t, sin_t = _rope_tables()
    scale = 1.0 / np.sqrt(DK)
    in_maps = []
    for c in range(NCORES):
        hs = slice(c * PD, (c + 1) * PD)
        biases = np.stack([
            np.asarray(bq)[hs] * scale, np.asarray(bk)[hs],
            np.asarray(bv)[hs]], axis=1).astype(np.float32)
        in_maps.append({
            "qH": qh, "kH": kh, "vH": vh,
            "wqH": _wh(np.asarray(Wq)[hs, :] * scale),
            "wkH": _wh(np.asarray(Wk)[hs, :]),
            "wvH": _wh(np.asarray(Wv)[hs, :]),
            "woT": np.ascontiguousarray(np.asarray(Wo)[:, hs].T).astype(bf),
            "biases": biases,
            "cos_t": cos_t, "sin_t": sin_t,
        })
    return in_maps


_NC_CACHE = {}


def _get_nc():
    if "nc" not in _NC_CACHE:
        nc = build_nc()
        nc.finalize()
        _NC_CACHE["nc"] = nc
    return _NC_CACHE["nc"]


def kernel(q, k, v, Wq, bq, Wk, bk, Wv, bv, Wo, bo):
    assert q.shape == (B, L, D) and k.shape == (B, L, D) and v.shape == (B, L, D)
    in_maps = host_in_maps(q, k, v, Wq, bq, Wk, bk, Wv, bv, Wo)
    nc = _get_nc()
    res = run_bass_kernel_spmd(nc, in_maps, list(range(NCORES)))
    out = np.zeros((TOK, D), np.float64)
    for r in res.results:
        out += r["outp"].astype(np.float64)
    out += np.asarray(bo, np.float64)[None, :]
    return out.astype(np.float32).reshape(B, L, D)
